# revision 21
# baseline (speedup 1.0000x reference)
"""DynaFormer (Graphormer-style GNN transformer) on 8 TRN2 NeuronCores.

Sharding: sequence-parallel over the N=2048 query rows (256/core).
Params replicated; K/V computed replicated from an all-gathered residual;
attention (scores/softmax/PV), Wo, LN2, FF row-sharded. Collectives:
1 AllReduce (degree histograms, 16KB) + 2 AllGathers (residual, 1MB).

Self-contained: hardcodes all shapes; builds + compiles + runs the Bass
kernel via run_bass_kernel_spmd on cores 0-7.
"""
import os
import numpy as np
import ml_dtypes

import concourse.bass as bass
import concourse.bacc as bacc
import concourse.tile as tile
import concourse.mybir as mybir
from concourse.bass_utils import run_bass_kernel_spmd
from concourse.masks import make_identity

dt = mybir.dt
F32 = dt.float32
BF16 = dt.bfloat16
I32 = dt.int32
AF = mybir.ActivationFunctionType
ALU = mybir.AluOpType

N, E, IN_NODE, D = 2048, 65536, 16, 128
H, FFD, HS, L = 8, 512, 8, 3
MAXD, OUT = 64, 64
NC = 8
RPC = N // NC            # 256 rows/core
EPC = E // NC            # 8192 edges/core
NT = N // 128            # 16 node tiles
SCALE = float(1.0 / np.sqrt(D))

# packed weight segment layout: name -> (count, width)
_BSEGS = [("wq", 24, 128), ("wk", 24, 128), ("wo", 24, 128), ("wv", 3, 1024),
          ("w1", 3, 512), ("w2", 12, 128), ("ow", 1, 64), ("bv", 24, 1)]
_FSEGS = [("bq", 24), ("bk", 24), ("l1g", 3), ("l1b", 3), ("l2g", 3), ("l2b", 3),
          ("b2c", 3), ("b1c", 12)]
BOFF = {}
_o = 0
for _n, _c, _w in _BSEGS:
    BOFF[_n] = _o
    _o += _c * _w
NBF = _o
FOFF = {}
_o = 0
for _n, _c in _FSEGS:
    FOFF[_n] = _o
    _o += _c
NF32 = _o

DBG = bool(int(os.environ.get("BASS_DBG", "0")))

_cached = {}


def bf16(x):
    return np.ascontiguousarray(np.asarray(x, np.float32).astype(ml_dtypes.bfloat16))


def f32(x):
    return np.ascontiguousarray(np.asarray(x, np.float32))


def build():
    nc = bacc.Bacc("TRN2", target_bir_lowering=False, debug=False,
                   enable_asserts=True, num_devices=NC)

    def din(name, shape, dty):
        return nc.dram_tensor(name, list(shape), dty, kind="ExternalInput").ap()

    def dout(name, shape, dty=F32):
        return nc.dram_tensor(name, list(shape), dty, kind="ExternalOutput").ap()

    # ---- dram params (common)
    xa_d = din("xa", [IN_NODE + 1, N], BF16)
    zin_d = din("zin", [MAXD, D], BF16)
    zout_d = din("zout", [MAXD, D], BF16)
    posT_d = din("posT", [3, N], BF16)
    wb_d = din("wb", [128, NBF], BF16)
    pf_d = din("pf", [128, NF32], F32)
    boall_d = din("boall", [1, L * D], F32)
    wina_d = din("wina", [IN_NODE + 1, D], BF16)
    obc_d = din("obc", [OUT, 1], F32)
    spr_d = din("spr", [1, 4 * HS], F32)   # mu | sigma | w | sp_b*8
    io64_d = din("iota64", [MAXD, 1], F32)
    # ---- per-core
    pcT_d = din("pcT", [3, RPC], BF16)
    src_d = din("src", [128, EPC // 128], I32)
    dst_d = din("dst", [128, EPC // 128], I32)
    oidx_d = din("oidx", [128, 2], I32)
    out_d = dout("out", [OUT, RPC], F32)
    dbg = {}
    if DBG:
        dbg["h0"] = dout("dbg_h0", [N, D])
        dbg["EB0"] = dout("dbg_EB0", [128, 512], BF16)
        dbg["xnT"] = dout("dbg_xnT", [D, N], BF16)
        dbg["KT0"] = dout("dbg_KT0", [D, N], BF16)
        dbg["QT0"] = dout("dbg_QT0", [D, RPC], BF16)
        dbg["PT0"] = dout("dbg_PT0", [128, RPC], BF16)
        dbg["OT0"] = dout("dbg_OT0", [D, RPC], BF16)
        dbg["rs0"] = dout("dbg_rs0", [1, RPC])
        dbg["hL0"] = dout("dbg_hL0", [RPC, D])
        dbg["cnt"] = dout("dbg_cnt", [1, 4096], BF16)

    with tile.TileContext(nc) as tc:
        with tc.tile_pool(name="cp", bufs=1) as cp, \
             tc.tile_pool(name="wp", bufs=3) as wp, \
             tc.tile_pool(name="pp", bufs=2, space="PSUM") as pp, \
             tc.tile_pool(name="dp", bufs=1, space="DRAM") as dp:

            dma = nc.sync.dma_start

            # ---------- persistent constants ----------
            ident = cp.tile([128, 128], BF16)
            make_identity(nc, ident[:])
            identf = cp.tile([128, 128], F32)
            make_identity(nc, identf[:])
            onesc_bf = cp.tile([128, 1], BF16)
            nc.vector.memset(onesc_bf[:], 1.0)
            eps5 = cp.tile([128, 1], F32)
            nc.vector.memset(eps5[:], 1e-5)

            WB = cp.tile([128, NBF], BF16)
            dma(WB[:], wb_d[:])
            PF = cp.tile([128, NF32], F32)
            dma(PF[:], pf_d[:])
            bo_all = cp.tile([1, L * D], F32)
            dma(bo_all[:], boall_d[:])
            obc = cp.tile([OUT, 1], F32)
            dma(obc[:], obc_d[:])

            def bseg(name, i, w):
                off = BOFF[name] + i * w
                return WB[:, off:off + w]

            def fseg(name, i):
                off = FOFF[name] + i
                return PF[:, off:off + 1]

            wq = [[bseg("wq", l * H + h, D) for h in range(H)] for l in range(L)]
            wk = [[bseg("wk", l * H + h, D) for h in range(H)] for l in range(L)]
            wo = [[bseg("wo", l * H + h, D) for h in range(H)] for l in range(L)]
            wv = [bseg("wv", l, H * D) for l in range(L)]
            w1 = [bseg("w1", l, FFD) for l in range(L)]
            w2 = [[bseg("w2", l * 4 + fs, D) for fs in range(4)] for l in range(L)]
            ow = bseg("ow", 0, OUT)
            bv = [[bseg("bv", l * H + h, 1) for h in range(H)] for l in range(L)]
            bq = [[fseg("bq", l * H + h) for h in range(H)] for l in range(L)]
            bk = [[fseg("bk", l * H + h) for h in range(H)] for l in range(L)]
            l1g = [fseg("l1g", l) for l in range(L)]
            l1b = [fseg("l1b", l) for l in range(L)]
            l2g = [fseg("l2g", l) for l in range(L)]
            l2b = [fseg("l2b", l) for l in range(L)]
            b2c = [fseg("b2c", l) for l in range(L)]
            b1c = [[fseg("b1c", l * 4 + fs) for fs in range(4)] for l in range(L)]
            bo = [bo_all[0:1, l * D:(l + 1) * D] for l in range(L)]

            # persistent activations
            hfull = [cp.tile([128, D], F32, name=f"hf{t}") for t in range(NT)]
            hown = [cp.tile([128, D], F32, name=f"ho{j}") for j in range(2)]
            EB = [cp.tile([128, 512], BF16, name=f"EB{kt}") for kt in range(NT)]
            xnT = cp.tile([D, N], BF16)
            xnTo = cp.tile([D, RPC], BF16)
            Vall = [cp.tile([128, H * D], BF16, name=f"V{t}") for t in range(NT)]
            QT = cp.tile([D, H * RPC], BF16)
            OT = cp.tile([D, H * RPC], BF16)
            rsr = cp.tile([1, H * RPC], F32)
            rrc = cp.tile([128, 2 * H], F32)
            xn2T = cp.tile([D, RPC], BF16)
            gT = [cp.tile([128, RPC], BF16, name=f"gT{fs}") for fs in range(4)]
            htmp_dram = dp.tile([N, D], F32)
            ar_in = dp.tile([32, 128], F32)
            ar_out = dp.tile([32, 128], F32)
            ag_in = [dp.tile([RPC, D], F32, name=f"agi{l}") for l in range(L - 1)]
            ag_out = [dp.tile([N, D], F32, name=f"ago{l}") for l in range(L - 1)]

            # ========================================================
            # PREPROC (scoped pool: tiles freed before the layer loop)
            # ========================================================
            with tc.tile_pool(name="prep", bufs=1) as prp:
                xa = prp.tile([IN_NODE + 1, N], BF16)
                dma(xa[:], xa_d[:])
                wina = prp.tile([IN_NODE + 1, D], BF16)
                dma(wina[:], wina_d[:])
                zin = prp.tile([MAXD, D], BF16)
                dma(zin[:], zin_d[:])
                zout = prp.tile([MAXD, D], BF16)
                dma(zout[:], zout_d[:])
                posT = prp.tile([3, N], BF16)
                dma(posT[:], posT_d[:])
                pcT = prp.tile([3, RPC], BF16)
                dma(pcT[:], pcT_d[:])
                io64 = prp.tile([MAXD, 1], F32)
                dma(io64[:], io64_d[:])
                spr = prp.tile([1, 4 * HS], F32)
                dma(spr[:], spr_d[:])
                oidx = prp.tile([128, 2], I32)
                dma(oidx[:], oidx_d[:])
                ones_bf = prp.tile([1, 128], BF16)
                nc.vector.memset(ones_bf[:], 1.0)
                ones_f = prp.tile([1, 128], F32)
                nc.vector.memset(ones_f[:], 1.0)
                ones3_f = prp.tile([3, 1], F32)
                nc.vector.memset(ones3_f[:], 1.0)
                eps12 = prp.tile([128, 1], F32)
                nc.vector.memset(eps12[:], 1e-12)

                # ---------- spatial bias -> EB = exp(b) duplicated [128, 512] ----------
                p5 = prp.tile([5, N], BF16)
                nc.vector.tensor_scalar(out=p5[0:3, :], in0=posT[:], scalar1=-2.0,
                                        scalar2=None, op0=ALU.mult)
                onesN = prp.tile([1, 512], BF16)
                nc.vector.memset(onesN[:], 1.0)
                for i in range(4):
                    dma(p5[4:5, i * 512:(i + 1) * 512], onesN[0:1, :])
                for i in range(4):
                    sq3 = wp.tile([3, 512], F32, tag="sq3", bufs=2)
                    nc.scalar.activation(sq3[:], posT[:, i * 512:(i + 1) * 512], AF.Square)
                    njp = pp.tile([1, 512], F32, space="PSUM", tag="psA")
                    nc.tensor.matmul(out=njp[:], lhsT=ones3_f[:3, 0:1],
                                     rhs=sq3[:], start=True, stop=True)
                    njrow = wp.tile([1, 512], BF16, tag="njrow", bufs=2)
                    nc.vector.tensor_copy(njrow[:], njp[:])
                    dma(p5[3:4, i * 512:(i + 1) * 512], njrow[0:1, :])
                r5 = prp.tile([5, RPC], BF16)
                nc.vector.tensor_copy(r5[0:3, :], pcT[:])
                dma(r5[3:4, :], onesN[0:1, 0:RPC])
                sqc = wp.tile([3, RPC], F32, tag="sqc", bufs=1)
                nc.scalar.activation(sqc[:], pcT[:], AF.Square)
                nqp = pp.tile([1, RPC], F32, space="PSUM", tag="psA")
                nc.tensor.matmul(out=nqp[:], lhsT=ones3_f[:3, 0:1], rhs=sqc[:],
                                 start=True, stop=True)
                nqrow = wp.tile([1, RPC], BF16, tag="nqrow", bufs=1)
                nc.vector.tensor_copy(nqrow[:], nqp[:])
                dma(r5[4:5, :], nqrow[0:1, :])

                # sp params broadcast to [128, HS] via K=1 matmuls
                spc = []
                for i in range(4):
                    sps = pp.tile([128, HS], F32, space="PSUM", tag="psA")
                    nc.tensor.matmul(out=sps[:], lhsT=ones_f[:1, :], rhs=spr[0:1, i * 8:i * 8 + 8],
                                     start=True, stop=True, skip_group_check=True)
                    t_ = prp.tile([128, HS], F32, name=f"spc{i}")
                    nc.vector.tensor_copy(t_[:], sps[:])
                    spc.append(t_)
                mu_bc, sg_bc, w_bc, b0_bc = spc
                rsig = prp.tile([128, HS], F32)
                nc.vector.reciprocal(rsig[:], sg_bc[:])
                msc = prp.tile([128, HS], F32)
                nc.vector.tensor_tensor(out=msc[:], in0=mu_bc[:], in1=rsig[:], op=ALU.mult)
                nc.vector.tensor_scalar(out=msc[:], in0=msc[:], scalar1=-1.0, scalar2=None,
                                        op0=ALU.mult)

                # all d-tiles first (one Sqrt table), then gaussians (Square/Exp set)
                dds = [prp.tile([128, RPC], BF16, name=f"dd{kt}") for kt in range(NT)]
                for kt in range(NT):
                    d2p = pp.tile([128, RPC], F32, space="PSUM", tag="psA")
                    nc.tensor.matmul(out=d2p[:], lhsT=p5[:, kt * 128:(kt + 1) * 128],
                                     rhs=r5[:], start=True, stop=True)
                    d2c = wp.tile([128, RPC], F32, tag="d2c")
                    nc.vector.tensor_scalar(out=d2c[:], in0=d2p[:], scalar1=0.0,
                                            scalar2=None, op0=ALU.max)
                    nc.scalar.activation(dds[kt][:], d2c[:], AF.Sqrt, bias=eps12[:, 0:1])
                for kt in range(NT):
                    bacc_t = wp.tile([128, RPC], BF16, tag="bacc")
                    for s in range(HS):
                        t1 = wp.tile([128, RPC], F32, tag="t1")
                        nc.scalar.activation(t1[:], dds[kt][:], AF.Square,
                                             bias=msc[:, s:s + 1], scale=rsig[:, s:s + 1])
                        g = wp.tile([128, RPC], BF16, tag="gga")
                        nc.scalar.activation(g[:], t1[:], AF.Exp, scale=-0.5)
                        if s == 0:
                            nc.vector.tensor_scalar(out=bacc_t[:], in0=g[:],
                                                    scalar1=w_bc[:, 0:1], scalar2=b0_bc[:, 0:1],
                                                    op0=ALU.mult, op1=ALU.add)
                        else:
                            gs = wp.tile([128, RPC], BF16, tag="gs")
                            nc.vector.tensor_scalar(out=gs[:], in0=g[:],
                                                    scalar1=w_bc[:, s:s + 1], scalar2=None,
                                                    op0=ALU.mult)
                            nc.gpsimd.tensor_tensor(out=bacc_t[:], in0=bacc_t[:], in1=gs[:],
                                                    op=ALU.add)
                    nc.scalar.activation(EB[kt][:, 0:RPC], bacc_t[:], AF.Exp)
                    nc.vector.tensor_copy(EB[kt][:, RPC:2 * RPC], EB[kt][:, 0:RPC])
                    if DBG and kt == 0:
                        dma(dbg["EB0"][:], EB[0][:])

                # ---------- degree histograms ----------
                iota128 = prp.tile([128, 128], I32)
                nc.gpsimd.iota(iota128[:], pattern=[[1, 128]], base=0, channel_multiplier=0)
                iota128f = prp.tile([128, 128], BF16)
                nc.vector.tensor_copy(iota128f[:], iota128[:])
                iota16 = prp.tile([128, 16], I32)
                nc.gpsimd.iota(iota16[:], pattern=[[1, 16]], base=0, channel_multiplier=0)
                iota16f = prp.tile([128, 16], BF16)
                nc.vector.tensor_copy(iota16f[:], iota16[:])

                cnts = [prp.tile([16, 128], F32, name=f"cnts{d}") for d in range(2)]
                NG = EPC // 128
                for di, ed in enumerate((src_d, dst_d)):
                    ei = wp.tile([128, NG], I32, tag="ei")
                    dma(ei[:], ed[:])
                    qi = wp.tile([128, NG], I32, tag="qi")
                    ri = wp.tile([128, NG], I32, tag="ri")
                    nc.vector.tensor_scalar(out=qi[:], in0=ei[:], scalar1=7, scalar2=None,
                                            op0=ALU.logical_shift_right)
                    nc.vector.tensor_scalar(out=ri[:], in0=ei[:], scalar1=127, scalar2=None,
                                            op0=ALU.bitwise_and)
                    qf = wp.tile([128, NG], BF16, tag="qf")
                    rf = wp.tile([128, NG], BF16, tag="rf")
                    nc.vector.tensor_copy(qf[:], qi[:])
                    nc.vector.tensor_copy(rf[:], ri[:])
                    hps = pp.tile([16, 128], F32, space="PSUM", tag="psA")
                    for g in range(NG):
                        Bg = wp.tile([128, 128], BF16, tag="Bg")
                        Ag = wp.tile([128, 16], BF16, tag="Ag")
                        nc.vector.tensor_tensor(out=Bg[:], in0=iota128f[:],
                                                in1=rf[:, g:g + 1].to_broadcast([128, 128]),
                                                op=ALU.is_equal)
                        nc.vector.tensor_tensor(out=Ag[:], in0=iota16f[:],
                                                in1=qf[:, g:g + 1].to_broadcast([128, 16]),
                                                op=ALU.is_equal)
                        nc.tensor.matmul(out=hps[:], lhsT=Ag[:], rhs=Bg[:],
                                         start=(g == 0), stop=(g == NG - 1))
                    nc.vector.tensor_copy(cnts[di][:], hps[:])

                dma(ar_in[0:16, :], cnts[0][:])
                dma(ar_in[16:32, :], cnts[1][:])
                nc.gpsimd.collective_compute(
                    "AllReduce", ALU.add, replica_groups=[list(range(NC))],
                    ins=[ar_in.opt()], outs=[ar_out.opt()])

                # ---------- h0 + degree embeds (replicated) ----------
                for t in range(NT):
                    hb = pp.tile([128, D], F32, space="PSUM", tag="psA")
                    nc.tensor.matmul(out=hb[:], lhsT=xa[:, t * 128:(t + 1) * 128],
                                     rhs=wina[:], start=True, stop=False,
                                     skip_group_check=True)
                    for di, ztab in ((0, zout), (1, zin)):
                        crow = wp.tile([1, 128], F32, tag="crow")
                        dma(crow[:], ar_out[di * 16 + t:di * 16 + t + 1, :])
                        crow16 = wp.tile([1, 128], BF16, tag="crow16")
                        nc.vector.tensor_scalar(out=crow16[:], in0=crow[:],
                                                scalar1=float(MAXD - 1), scalar2=None,
                                                op0=ALU.min)
                        if DBG:
                            dma(dbg["cnt"][0:1, (di * 16 + t) * 128:(di * 16 + t + 1) * 128],
                                crow16[0:1, :])
                        bc = pp.tile([MAXD, 128], F32, space="PSUM", tag="psA")
                        nc.tensor.matmul(out=bc[:], lhsT=ones_bf[:1, :MAXD],
                                         rhs=crow16[0:1, :],
                                         start=True, stop=True)
                        oh = wp.tile([MAXD, 128], BF16, tag="oh")
                        nc.vector.tensor_scalar(out=oh[:], in0=bc[:], scalar1=io64[:, 0:1],
                                                scalar2=None, op0=ALU.is_equal)
                        nc.tensor.matmul(out=hb[:], lhsT=oh[:], rhs=ztab[:],
                                         start=False, stop=(di == 1),
                                         skip_group_check=True)
                    nc.vector.tensor_copy(hfull[t][:], hb[:])
                    dma(htmp_dram[t * 128:(t + 1) * 128, :], hfull[t][:])
                    if DBG:
                        dma(dbg["h0"][t * 128:(t + 1) * 128, :], hfull[t][:])
                for j in range(2):
                    nc.gpsimd.indirect_dma_start(
                        out=hown[j][:], out_offset=None, in_=htmp_dram[:],
                        in_offset=bass.IndirectOffsetOnAxis(ap=oidx[:, j:j + 1], axis=0))

            # ========================================================
            # LAYERS
            # ========================================================
            def ln_group(srcs, gcol, bcol, outs):
                """Batched LayerNorm: stats for all tiles, one Sqrt + one
                reciprocal, then per-tile normalize + transpose + g/b evict."""
                nT = len(srcs)
                var = wp.tile([128, NT + 2], F32, tag="lnvar")
                mean = wp.tile([128, NT + 2], F32, tag="lnmean")
                for i, s_ap in enumerate(srcs):
                    st6 = wp.tile([128, 6], F32, tag="st6")
                    nc.vector.bn_stats(out=st6[:], in_=s_ap)
                    mv = wp.tile([128, 2], F32, tag="mv")
                    nc.vector.bn_aggr(out=mv[:], in_=st6[:])
                    nc.vector.tensor_copy(mean[:, i:i + 1], mv[:, 0:1])
                    nc.vector.tensor_copy(var[:, i:i + 1], mv[:, 1:2])
                sd = wp.tile([128, NT + 2], F32, tag="lnsd")
                nc.scalar.activation(sd[:, :nT], var[:, :nT], AF.Sqrt, bias=eps5[:, 0:1])
                rstd = wp.tile([128, NT + 2], F32, tag="lnrstd")
                nc.vector.reciprocal(rstd[:, :nT], sd[:, :nT])
                for i, s_ap in enumerate(srcs):
                    xn = wp.tile([128, D], BF16, tag="xn")
                    nc.vector.tensor_scalar(out=xn[:], in0=s_ap, scalar1=mean[:, i:i + 1],
                                            scalar2=rstd[:, i:i + 1], op0=ALU.subtract,
                                            op1=ALU.mult)
                    trp = pp.tile([128, 128], BF16, space="PSUM", tag="psA")
                    nc.tensor.transpose(out=trp[:], in_=xn[:], identity=ident[:])
                    nc.scalar.activation(outs[i], trp[:], AF.Identity,
                                         bias=bcol, scale=gcol)

            for l in range(L):
                ln_group([hfull[t][:] for t in range(NT)] + [hown[j][:] for j in range(2)],
                         l1g[l], l1b[l],
                         [xnT[:, t * 128:(t + 1) * 128] for t in range(NT)]
                         + [xnTo[:, j * 128:(j + 1) * 128] for j in range(2)])
                if DBG and l == 0:
                    dma(dbg["xnT"][:], xnT[:])
                # V (all heads) per n-tile; evictions on GpSimd (idle engine)
                for t in range(NT):
                    for i in range(2):
                        vp = pp.tile([128, 512], F32, space="PSUM", tag="psA")
                        nc.tensor.matmul(out=vp[:],
                                         lhsT=xnT[:, t * 128:(t + 1) * 128],
                                         rhs=wv[l][:, i * 512:(i + 1) * 512],
                                         start=True, stop=True)
                        nc.scalar.activation(Vall[t][:, i * 512:(i + 1) * 512], vp[:],
                                             AF.Identity)
                # Q^T per head
                for h in range(H):
                    qp = pp.tile([D, RPC], F32, space="PSUM", tag="psA")
                    nc.tensor.matmul(out=qp[:], lhsT=wq[l][h], rhs=xnTo[:],
                                     start=True, stop=True)
                    nc.vector.tensor_scalar(out=QT[:, h * RPC:(h + 1) * RPC],
                                            in0=qp[:], scalar1=bq[l][h],
                                            scalar2=None, op0=ALU.add)
                # bo' = bo + sum_h bv_h @ Wo_h
                bop_ps = pp.tile([1, D], F32, space="PSUM", tag="psA")
                for h in range(H):
                    nc.tensor.matmul(out=bop_ps[:], lhsT=bv[l][h],
                                     rhs=wo[l][h], start=(h == 0), stop=(h == H - 1))
                bop = cp.tile([1, D], F32, name=f"bop{l}")
                nc.vector.tensor_tensor(out=bop[:], in0=bop_ps[:], in1=bo[l],
                                        op=ALU.add)
                # flash attention, two heads per pass
                hpt = [wp.tile([128, D], F32, name=f"hp{j}") for j in range(2)]
                accD = [wp.tile([128, D], F32, name=f"accD{j}") for j in range(2)]
                for hp2 in range(H // 2):
                    h0, h1 = 2 * hp2, 2 * hp2 + 1
                    KTa = wp.tile([D, N], BF16, tag="KTa", bufs=2)
                    KTb = wp.tile([D, N], BF16, tag="KTb", bufs=2)
                    for hh, KT in ((h0, KTa), (h1, KTb)):
                        for i in range(4):
                            kp = pp.tile([D, 512], F32, space="PSUM", tag="psA")
                            nc.tensor.matmul(out=kp[:], lhsT=wk[l][hh],
                                             rhs=xnT[:, i * 512:(i + 1) * 512],
                                             start=True, stop=True)
                            nc.vector.tensor_scalar(out=KT[:, i * 512:(i + 1) * 512],
                                                    in0=kp[:], scalar1=bk[l][hh],
                                                    scalar2=None, op0=ALU.add)
                    otp0 = pp.tile([D, RPC], F32, space="PSUM", tag="otp", bufs=1)
                    otp1 = pp.tile([D, RPC], F32, space="PSUM", tag="otp2", bufs=1)
                    rsp = pp.tile([1, 512], F32, space="PSUM", tag="rsp", bufs=1)
                    for kt in range(NT):
                        stp = pp.tile([128, 512], F32, space="PSUM", tag="stp", bufs=3)
                        nc.tensor.matmul(out=stp[:, 0:RPC],
                                         lhsT=KTa[:, kt * 128:(kt + 1) * 128],
                                         rhs=QT[:, h0 * RPC:(h0 + 1) * RPC],
                                         start=True, stop=True, skip_group_check=True)
                        nc.tensor.matmul(out=stp[:, RPC:2 * RPC],
                                         lhsT=KTb[:, kt * 128:(kt + 1) * 128],
                                         rhs=QT[:, h1 * RPC:(h1 + 1) * RPC],
                                         start=True, stop=True, skip_group_check=True)
                        PT = wp.tile([128, 512], BF16, tag="PT")
                        nc.scalar.activation(PT[:], stp[:], AF.Exp, scale=SCALE)
                        nc.gpsimd.tensor_tensor(out=PT[:], in0=PT[:], in1=EB[kt][:],
                                                op=ALU.mult)
                        nc.tensor.matmul(out=rsp[:], lhsT=onesc_bf[:, 0:1], rhs=PT[:],
                                         start=(kt == 0), stop=(kt == NT - 1))
                        nc.tensor.matmul(out=otp0[:],
                                         lhsT=Vall[kt][:, h0 * D:(h0 + 1) * D],
                                         rhs=PT[:, 0:RPC],
                                         start=(kt == 0), stop=(kt == NT - 1))
                        nc.tensor.matmul(out=otp1[:],
                                         lhsT=Vall[kt][:, h1 * D:(h1 + 1) * D],
                                         rhs=PT[:, RPC:2 * RPC],
                                         start=(kt == 0), stop=(kt == NT - 1))
                        if DBG and l == 0 and hp2 == 0 and kt == 0:
                            dma(dbg["PT0"][:], PT[:, 0:RPC])
                    nc.vector.tensor_copy(OT[:, h0 * RPC:(h0 + 1) * RPC], otp0[:])
                    nc.vector.tensor_copy(OT[:, h1 * RPC:(h1 + 1) * RPC], otp1[:])
                    nc.vector.tensor_copy(rsr[:, hp2 * 512:(hp2 + 1) * 512], rsp[:])
                    if DBG and l == 0 and hp2 == 0:
                        dma(dbg["KT0"][:], KTa[:])
                        dma(dbg["QT0"][:], QT[:, 0:RPC])
                        dma(dbg["OT0"][:], OT[:, 0:RPC])
                        dma(dbg["rs0"][:], rsr[:, 0:RPC])
                rst = wp.tile([128, 2 * H], F32, tag="rst")
                for h in range(H):
                    for j in range(2):
                        rtp = pp.tile([128, 1], F32, space="PSUM", tag="psA")
                        nc.tensor.transpose(
                            out=rtp[:], in_=rsr[0:1, h * RPC + j * 128:h * RPC + (j + 1) * 128],
                            identity=identf[0:1, 0:1])
                        nc.vector.tensor_copy(rst[:, h * 2 + j:h * 2 + j + 1], rtp[:])
                nc.vector.reciprocal(rrc[:], rst[:])
                for j in range(2):
                    for h in range(H):
                        wdp = pp.tile([128, D], F32, space="PSUM", tag="psA")
                        if h == 0:
                            nc.tensor.matmul(out=wdp[:],
                                             lhsT=rsr[0:1, j * 128:(j + 1) * 128],
                                             rhs=bop[:], start=True, stop=False,
                                             skip_group_check=True)
                        nc.tensor.matmul(out=wdp[:],
                                         lhsT=OT[:, h * RPC + j * 128:h * RPC + (j + 1) * 128],
                                         rhs=wo[l][h], start=(h != 0), stop=True,
                                         skip_group_check=True)
                        if h == 0:
                            nc.vector.tensor_scalar(out=accD[j][:], in0=wdp[:],
                                                    scalar1=rrc[:, j:j + 1],
                                                    scalar2=None, op0=ALU.mult)
                        else:
                            tmpw = wp.tile([128, D], F32, tag="tmpw")
                            nc.vector.tensor_scalar(out=tmpw[:], in0=wdp[:],
                                                    scalar1=rrc[:, h * 2 + j:h * 2 + j + 1],
                                                    scalar2=None, op0=ALU.mult)
                            nc.vector.tensor_tensor(out=accD[j][:], in0=accD[j][:],
                                                    in1=tmpw[:], op=ALU.add)
                    nc.vector.tensor_tensor(out=hpt[j][:], in0=hown[j][:],
                                            in1=accD[j][:], op=ALU.add)
                # LN2 + FF on own rows
                ln_group([hpt[j][:] for j in range(2)], l2g[l], l2b[l],
                         [xn2T[:, j * 128:(j + 1) * 128] for j in range(2)])
                for fs in range(4):
                    fp = pp.tile([128, RPC], F32, space="PSUM", tag="psA")
                    nc.tensor.matmul(out=fp[:], lhsT=w1[l][:, fs * 128:(fs + 1) * 128],
                                     rhs=xn2T[:], start=True, stop=True)
                    nc.scalar.activation(gT[fs][:], fp[:],
                                         AF.Gelu, bias=b1c[l][fs])
                fdp = pp.tile([D, RPC], F32, space="PSUM", tag="psA")
                for fs in range(4):
                    nc.tensor.matmul(out=fdp[:], lhsT=w2[l][fs],
                                     rhs=gT[fs][:],
                                     start=(fs == 0), stop=(fs == 3))
                ffdT = wp.tile([D, RPC], BF16, tag="ffdT")
                nc.scalar.activation(ffdT[:], fdp[:], AF.Identity, bias=b2c[l])
                for j in range(2):
                    ftp = pp.tile([128, 128], BF16, space="PSUM", tag="psA")
                    nc.tensor.transpose(out=ftp[:], in_=ffdT[:, j * 128:(j + 1) * 128],
                                        identity=ident[:])
                    nc.vector.tensor_tensor(out=hown[j][:], in0=hpt[j][:], in1=ftp[:],
                                            op=ALU.add)
                if DBG and l == 0:
                    for j in range(2):
                        dma(dbg["hL0"][j * 128:(j + 1) * 128, :], hown[j][:])
                # AllGather residual (not needed after last layer)
                if l < L - 1:
                    for j in range(2):
                        dma(ag_in[l][j * 128:(j + 1) * 128, :], hown[j][:])
                    nc.gpsimd.collective_compute(
                        "AllGather", ALU.bypass, replica_groups=[list(range(NC))],
                        ins=[ag_in[l].opt()], outs=[ag_out[l].opt()])
                    for t in range(NT):
                        dma(hfull[t][:], ag_out[l][t * 128:(t + 1) * 128, :])

            # ================= output =================
            for j in range(2):
                hb16 = wp.tile([128, D], BF16, tag="hb16")
                nc.vector.tensor_copy(hb16[:], hown[j][:])
                htp = pp.tile([128, 128], BF16, space="PSUM", tag="psA")
                nc.tensor.transpose(out=htp[:], in_=hb16[:], identity=ident[:])
                hT = wp.tile([D, 128], BF16, tag="hT")
                nc.vector.tensor_copy(hT[:], htp[:])
                op_ps = pp.tile([OUT, 128], F32, space="PSUM", tag="psA")
                nc.tensor.matmul(out=op_ps[:], lhsT=ow, rhs=hT[:],
                                 start=True, stop=True)
                ob_sb = wp.tile([OUT, 128], F32, tag="ob_sb")
                nc.scalar.activation(ob_sb[:], op_ps[:], AF.Identity,
                                     bias=obc[:, 0:1])
                dma(out_d[:, j * 128:(j + 1) * 128], ob_sb[:])

    nc.finalize()
    return nc


def _pack_bf16(I):
    Wq, Wk, Wo_, Wv_ = f32(I["Wq"]), f32(I["Wk"]), f32(I["Wo"]), f32(I["Wv"])
    cols = []
    for l in range(L):
        for h in range(H):
            cols.append(Wq[l, h])
    for l in range(L):
        for h in range(H):
            cols.append(Wk[l, h])
    for l in range(L):
        for h in range(H):
            cols.append(Wo_[l, h * D:(h + 1) * D, :])
    for l in range(L):
        cols.append(Wv_[l].transpose(1, 0, 2).reshape(D, H * D))
    for l in range(L):
        cols.append(f32(I["ff1_w"])[l])
    for l in range(L):
        for fs in range(4):
            cols.append(f32(I["ff2_w"])[l, fs * 128:(fs + 1) * 128, :])
    cols.append(f32(I["out_w"]))
    bvv = f32(I["bv"])
    for l in range(L):
        for h in range(H):
            cols.append(bvv[l, h][:, None])
    out = np.concatenate(cols, 1)
    assert out.shape == (128, NBF), out.shape
    return bf16(out)


def _pack_f32(I):
    cols = []
    for l in range(L):
        for h in range(H):
            cols.append(f32(I["bq"])[l, h][:, None])
    for l in range(L):
        for h in range(H):
            cols.append(f32(I["bk"])[l, h][:, None])
    for nm in ("ln1_g", "ln1_b", "ln2_g", "ln2_b"):
        for l in range(L):
            cols.append(f32(I[nm])[l][:, None])
    for l in range(L):
        cols.append(f32(I["ff2_b"])[l][:, None])
    for l in range(L):
        for fs in range(4):
            cols.append(f32(I["ff1_b"])[l, fs * 128:(fs + 1) * 128][:, None])
    out = np.concatenate(cols, 1)
    assert out.shape == (128, NF32), out.shape
    return f32(out)


def _prep(inputs):
    I = {k: np.asarray(v) for k, v in inputs.items()}
    x = f32(I["x"])
    pos = f32(I["pos"])
    ei = I["edge_index"].astype(np.int32)
    Wv = f32(I["Wv"])

    common = {
        "xa": bf16(np.concatenate([x.T, np.ones((1, N), np.float32)], 0)),
        "wina": bf16(np.concatenate([f32(I["node_in_w"]), f32(I["node_in_b"])[None]], 0)),
        "zin": bf16(I["z_in"]),
        "zout": bf16(I["z_out"]),
        "posT": bf16(pos.T),
        "wb": _pack_bf16(I),
        "pf": _pack_f32(I),
        "boall": f32(I["bo"]).reshape(1, L * D),
        "obc": f32(I["out_b"])[:, None],
        "spr": np.concatenate([f32(I["sp_mu"]), f32(I["sp_sigma"]), f32(I["sp_w"]),
                               np.full(HS, float(I["sp_b"]), np.float32)])[None].astype(np.float32),
        "iota64": np.arange(MAXD, dtype=np.float32)[:, None],
    }
    common = {k: np.ascontiguousarray(v) for k, v in common.items()}
    in_maps = []
    for c in range(NC):
        m = dict(common)
        qsl = slice(c * RPC, (c + 1) * RPC)
        m["pcT"] = bf16(pos[qsl].T)
        m["src"] = np.ascontiguousarray(
            ei[0, c * EPC:(c + 1) * EPC].reshape(EPC // 128, 128).T)
        m["dst"] = np.ascontiguousarray(
            ei[1, c * EPC:(c + 1) * EPC].reshape(EPC // 128, 128).T)
        m["oidx"] = np.ascontiguousarray(
            (c * RPC + np.arange(RPC, dtype=np.int32)).reshape(2, 128).T)
        in_maps.append(m)
    return in_maps


def kernel(**inputs) -> np.ndarray:
    if "nc" not in _cached:
        _cached["nc"] = build()
    in_maps = _prep(inputs)
    res = run_bass_kernel_spmd(_cached["nc"], in_maps, core_ids=list(range(NC)))
    _cached["last_results"] = res
    out = np.concatenate([f32(r["out"]).T for r in res.results], 0)
    return out.astype(np.float32)


# revision 22
# speedup vs baseline: 1.1053x; 1.1053x over previous
"""DynaFormer (Graphormer-style GNN transformer) on 8 TRN2 NeuronCores.

Sharding: sequence-parallel over the N=2048 query rows (256/core).
Params replicated; K/V computed replicated from an all-gathered residual;
attention (scores/softmax/PV), Wo, LN2, FF row-sharded. Collectives:
1 AllReduce (degree histograms, 16KB) + 2 AllGathers (residual, 1MB).

Self-contained: hardcodes all shapes; builds + compiles + runs the Bass
kernel via run_bass_kernel_spmd on cores 0-7.
"""
import os
import numpy as np
import ml_dtypes

import concourse.bass as bass
import concourse.bacc as bacc
import concourse.tile as tile
import concourse.mybir as mybir
from concourse.bass_utils import run_bass_kernel_spmd
from concourse.masks import make_identity

dt = mybir.dt
F32 = dt.float32
BF16 = dt.bfloat16
I32 = dt.int32
AF = mybir.ActivationFunctionType
ALU = mybir.AluOpType

N, E, IN_NODE, D = 2048, 65536, 16, 128
H, FFD, HS, L = 8, 512, 8, 3
MAXD, OUT = 64, 64
NC = 8
RPC = N // NC            # 256 rows/core
EPC = E // NC            # 8192 edges/core
NT = N // 128            # 16 node tiles
SCALE = float(1.0 / np.sqrt(D))

# packed weight segment layout: name -> (count, width)
_BSEGS = [("wq", 24, 128), ("wk", 24, 128), ("wo", 24, 128), ("wv", 3, 1024),
          ("w1", 3, 512), ("w2", 12, 128), ("ow", 1, 64), ("bv", 24, 1)]
_FSEGS = [("bq", 24), ("bk", 24), ("l1g", 3), ("l1b", 3), ("l2g", 3), ("l2b", 3),
          ("b2c", 3), ("b1c", 12)]
BOFF = {}
_o = 0
for _n, _c, _w in _BSEGS:
    BOFF[_n] = _o
    _o += _c * _w
NBF = _o
FOFF = {}
_o = 0
for _n, _c in _FSEGS:
    FOFF[_n] = _o
    _o += _c
NF32 = _o

DBG = bool(int(os.environ.get("BASS_DBG", "0")))

_cached = {}


def bf16(x):
    return np.ascontiguousarray(np.asarray(x, np.float32).astype(ml_dtypes.bfloat16))


def f32(x):
    return np.ascontiguousarray(np.asarray(x, np.float32))


def build():
    nc = bacc.Bacc("TRN2", target_bir_lowering=False, debug=False,
                   enable_asserts=True, num_devices=NC)

    def din(name, shape, dty):
        return nc.dram_tensor(name, list(shape), dty, kind="ExternalInput").ap()

    def dout(name, shape, dty=F32):
        return nc.dram_tensor(name, list(shape), dty, kind="ExternalOutput").ap()

    # ---- dram params (common)
    xa_d = din("xa", [IN_NODE + 1, N], BF16)
    zin_d = din("zin", [MAXD, D], BF16)
    zout_d = din("zout", [MAXD, D], BF16)
    posT_d = din("posT", [3, N], BF16)
    wb_d = din("wb", [128, NBF], BF16)
    pf_d = din("pf", [128, NF32], F32)
    boall_d = din("boall", [1, L * D], F32)
    wina_d = din("wina", [IN_NODE + 1, D], BF16)
    obc_d = din("obc", [OUT, 1], F32)
    spr_d = din("spr", [1, 4 * HS], F32)   # mu | sigma | w | sp_b*8
    io64_d = din("iota64", [MAXD, 1], F32)
    # ---- per-core
    pcT_d = din("pcT", [3, RPC], BF16)
    src_d = din("src", [128, EPC // 128], I32)
    dst_d = din("dst", [128, EPC // 128], I32)
    oidx_d = din("oidx", [128, 2], I32)
    out_d = dout("out", [OUT, RPC], F32)
    dbg = {}
    if DBG:
        dbg["h0"] = dout("dbg_h0", [N, D])
        dbg["EB0"] = dout("dbg_EB0", [128, 512], BF16)
        dbg["xnT"] = dout("dbg_xnT", [D, N], BF16)
        dbg["KT0"] = dout("dbg_KT0", [D, N], BF16)
        dbg["QT0"] = dout("dbg_QT0", [D, RPC], BF16)
        dbg["PT0"] = dout("dbg_PT0", [128, RPC], BF16)
        dbg["OT0"] = dout("dbg_OT0", [D, RPC], BF16)
        dbg["rs0"] = dout("dbg_rs0", [1, RPC])
        dbg["hL0"] = dout("dbg_hL0", [RPC, D])
        dbg["cnt"] = dout("dbg_cnt", [1, 4096], BF16)

    with tile.TileContext(nc) as tc:
        with tc.tile_pool(name="cp", bufs=1) as cp, \
             tc.tile_pool(name="wp", bufs=3) as wp, \
             tc.tile_pool(name="pp", bufs=2, space="PSUM") as pp, \
             tc.tile_pool(name="dp", bufs=1, space="DRAM") as dp:

            dma = nc.sync.dma_start

            # ---------- persistent constants ----------
            ident = cp.tile([128, 128], BF16)
            make_identity(nc, ident[:])
            identf = cp.tile([128, 128], F32)
            make_identity(nc, identf[:])
            onesc_bf = cp.tile([128, 1], BF16)
            nc.vector.memset(onesc_bf[:], 1.0)
            eps5 = cp.tile([128, 1], F32)
            nc.vector.memset(eps5[:], 1e-5)

            WB = cp.tile([128, NBF], BF16)
            dma(WB[:], wb_d[:])
            PF = cp.tile([128, NF32], F32)
            dma(PF[:], pf_d[:])
            bo_all = cp.tile([1, L * D], F32)
            dma(bo_all[:], boall_d[:])
            obc = cp.tile([OUT, 1], F32)
            dma(obc[:], obc_d[:])

            def bseg(name, i, w):
                off = BOFF[name] + i * w
                return WB[:, off:off + w]

            def fseg(name, i):
                off = FOFF[name] + i
                return PF[:, off:off + 1]

            wq = [[bseg("wq", l * H + h, D) for h in range(H)] for l in range(L)]
            wk = [[bseg("wk", l * H + h, D) for h in range(H)] for l in range(L)]
            wo = [[bseg("wo", l * H + h, D) for h in range(H)] for l in range(L)]
            wv = [bseg("wv", l, H * D) for l in range(L)]
            w1 = [bseg("w1", l, FFD) for l in range(L)]
            w2 = [[bseg("w2", l * 4 + fs, D) for fs in range(4)] for l in range(L)]
            ow = bseg("ow", 0, OUT)
            bv = [[bseg("bv", l * H + h, 1) for h in range(H)] for l in range(L)]
            bq = [[fseg("bq", l * H + h) for h in range(H)] for l in range(L)]
            bk = [[fseg("bk", l * H + h) for h in range(H)] for l in range(L)]
            l1g = [fseg("l1g", l) for l in range(L)]
            l1b = [fseg("l1b", l) for l in range(L)]
            l2g = [fseg("l2g", l) for l in range(L)]
            l2b = [fseg("l2b", l) for l in range(L)]
            b2c = [fseg("b2c", l) for l in range(L)]
            b1c = [[fseg("b1c", l * 4 + fs) for fs in range(4)] for l in range(L)]
            bo = [bo_all[0:1, l * D:(l + 1) * D] for l in range(L)]

            # persistent activations
            hfull = [cp.tile([128, D], F32, name=f"hf{t}") for t in range(NT)]
            hown = [cp.tile([128, D], F32, name=f"ho{j}") for j in range(2)]
            EB = [cp.tile([128, 512], BF16, name=f"EB{kt}") for kt in range(NT)]
            xnT = cp.tile([D, N], BF16)
            xnTo = cp.tile([D, RPC], BF16)
            Vall = [cp.tile([128, H * D], BF16, name=f"V{t}") for t in range(NT)]
            QT = cp.tile([D, H * RPC], BF16)
            OT = cp.tile([D, H * RPC], BF16)
            rsr = cp.tile([1, H * RPC], F32)
            rrc = cp.tile([128, 2 * H], F32)
            xn2T = cp.tile([D, RPC], BF16)
            gT = [cp.tile([128, RPC], BF16, name=f"gT{fs}") for fs in range(4)]
            htmp_dram = dp.tile([N, D], F32)
            ar_in = dp.tile([32, 128], F32)
            ar_out = dp.tile([32, 128], F32)
            ag_in = [dp.tile([RPC, D], F32, name=f"agi{l}") for l in range(L - 1)]
            ag_out = [dp.tile([N, D], F32, name=f"ago{l}") for l in range(L - 1)]

            # ========================================================
            # PREPROC (scoped pool: tiles freed before the layer loop)
            # ========================================================
            with tc.tile_pool(name="prep", bufs=1) as prp:
                xa = prp.tile([IN_NODE + 1, N], BF16)
                dma(xa[:], xa_d[:])
                wina = prp.tile([IN_NODE + 1, D], BF16)
                dma(wina[:], wina_d[:])
                zin = prp.tile([MAXD, D], BF16)
                dma(zin[:], zin_d[:])
                zout = prp.tile([MAXD, D], BF16)
                dma(zout[:], zout_d[:])
                posT = prp.tile([3, N], BF16)
                dma(posT[:], posT_d[:])
                pcT = prp.tile([3, RPC], BF16)
                dma(pcT[:], pcT_d[:])
                io64 = prp.tile([MAXD, 1], F32)
                dma(io64[:], io64_d[:])
                spr = prp.tile([1, 4 * HS], F32)
                dma(spr[:], spr_d[:])
                oidx = prp.tile([128, 2], I32)
                dma(oidx[:], oidx_d[:])
                ones_bf = prp.tile([1, 128], BF16)
                nc.vector.memset(ones_bf[:], 1.0)
                ones_f = prp.tile([1, 128], F32)
                nc.vector.memset(ones_f[:], 1.0)
                ones3_f = prp.tile([3, 1], F32)
                nc.vector.memset(ones3_f[:], 1.0)
                eps12 = prp.tile([128, 1], F32)
                nc.vector.memset(eps12[:], 1e-12)

                # ---------- spatial bias -> EB = exp(b) duplicated [128, 512] ----------
                p5 = prp.tile([5, N], BF16)
                nc.vector.tensor_scalar(out=p5[0:3, :], in0=posT[:], scalar1=-2.0,
                                        scalar2=None, op0=ALU.mult)
                onesN = prp.tile([1, 512], BF16)
                nc.vector.memset(onesN[:], 1.0)
                for i in range(4):
                    dma(p5[4:5, i * 512:(i + 1) * 512], onesN[0:1, :])
                for i in range(4):
                    sq3 = wp.tile([3, 512], F32, tag="sq3", bufs=2)
                    nc.scalar.activation(sq3[:], posT[:, i * 512:(i + 1) * 512], AF.Square)
                    njp = pp.tile([1, 512], F32, space="PSUM", tag="psA")
                    nc.tensor.matmul(out=njp[:], lhsT=ones3_f[:3, 0:1],
                                     rhs=sq3[:], start=True, stop=True)
                    njrow = wp.tile([1, 512], BF16, tag="njrow", bufs=2)
                    nc.vector.tensor_copy(njrow[:], njp[:])
                    dma(p5[3:4, i * 512:(i + 1) * 512], njrow[0:1, :])
                r5 = prp.tile([5, RPC], BF16)
                nc.vector.tensor_copy(r5[0:3, :], pcT[:])
                dma(r5[3:4, :], onesN[0:1, 0:RPC])
                sqc = wp.tile([3, RPC], F32, tag="sqc", bufs=1)
                nc.scalar.activation(sqc[:], pcT[:], AF.Square)
                nqp = pp.tile([1, RPC], F32, space="PSUM", tag="psA")
                nc.tensor.matmul(out=nqp[:], lhsT=ones3_f[:3, 0:1], rhs=sqc[:],
                                 start=True, stop=True)
                nqrow = wp.tile([1, RPC], BF16, tag="nqrow", bufs=1)
                nc.vector.tensor_copy(nqrow[:], nqp[:])
                dma(r5[4:5, :], nqrow[0:1, :])

                # sp params broadcast to [128, HS] via K=1 matmuls
                spc = []
                for i in range(4):
                    sps = pp.tile([128, HS], F32, space="PSUM", tag="psA")
                    nc.tensor.matmul(out=sps[:], lhsT=ones_f[:1, :], rhs=spr[0:1, i * 8:i * 8 + 8],
                                     start=True, stop=True, skip_group_check=True)
                    t_ = prp.tile([128, HS], F32, name=f"spc{i}")
                    nc.vector.tensor_copy(t_[:], sps[:])
                    spc.append(t_)
                mu_bc, sg_bc, w_bc, b0_bc = spc
                rsig = prp.tile([128, HS], F32)
                nc.vector.reciprocal(rsig[:], sg_bc[:])
                msc = prp.tile([128, HS], F32)
                nc.vector.tensor_tensor(out=msc[:], in0=mu_bc[:], in1=rsig[:], op=ALU.mult)
                nc.vector.tensor_scalar(out=msc[:], in0=msc[:], scalar1=-1.0, scalar2=None,
                                        op0=ALU.mult)

                # all d-tiles first (one Sqrt table), then gaussians (Square/Exp set)
                dds = [prp.tile([128, RPC], BF16, name=f"dd{kt}") for kt in range(NT)]
                for kt in range(NT):
                    d2p = pp.tile([128, RPC], F32, space="PSUM", tag="psA")
                    nc.tensor.matmul(out=d2p[:], lhsT=p5[:, kt * 128:(kt + 1) * 128],
                                     rhs=r5[:], start=True, stop=True)
                    d2c = wp.tile([128, RPC], F32, tag="d2c")
                    nc.vector.tensor_scalar(out=d2c[:], in0=d2p[:], scalar1=0.0,
                                            scalar2=None, op0=ALU.max)
                    nc.scalar.activation(dds[kt][:], d2c[:], AF.Sqrt, bias=eps12[:, 0:1])
                for kt in range(NT):
                    bacc_t = wp.tile([128, RPC], BF16, tag="bacc")
                    for s in range(HS):
                        t1 = wp.tile([128, RPC], F32, tag="t1")
                        nc.scalar.activation(t1[:], dds[kt][:], AF.Square,
                                             bias=msc[:, s:s + 1], scale=rsig[:, s:s + 1])
                        g = wp.tile([128, RPC], BF16, tag="gga")
                        nc.scalar.activation(g[:], t1[:], AF.Exp, scale=-0.5)
                        if s == 0:
                            nc.vector.tensor_scalar(out=bacc_t[:], in0=g[:],
                                                    scalar1=w_bc[:, 0:1], scalar2=b0_bc[:, 0:1],
                                                    op0=ALU.mult, op1=ALU.add)
                        else:
                            gs = wp.tile([128, RPC], BF16, tag="gs")
                            nc.vector.tensor_scalar(out=gs[:], in0=g[:],
                                                    scalar1=w_bc[:, s:s + 1], scalar2=None,
                                                    op0=ALU.mult)
                            nc.gpsimd.tensor_tensor(out=bacc_t[:], in0=bacc_t[:], in1=gs[:],
                                                    op=ALU.add)
                    nc.scalar.activation(EB[kt][:, 0:RPC], bacc_t[:], AF.Exp)
                    nc.vector.tensor_copy(EB[kt][:, RPC:2 * RPC], EB[kt][:, 0:RPC])
                    if DBG and kt == 0:
                        dma(dbg["EB0"][:], EB[0][:])

                # ---------- degree histograms ----------
                iota128 = prp.tile([128, 128], I32)
                nc.gpsimd.iota(iota128[:], pattern=[[1, 128]], base=0, channel_multiplier=0)
                iota128f = prp.tile([128, 128], BF16)
                nc.vector.tensor_copy(iota128f[:], iota128[:])
                iota16 = prp.tile([128, 16], I32)
                nc.gpsimd.iota(iota16[:], pattern=[[1, 16]], base=0, channel_multiplier=0)
                iota16f = prp.tile([128, 16], BF16)
                nc.vector.tensor_copy(iota16f[:], iota16[:])

                cnts = [prp.tile([16, 128], F32, name=f"cnts{d}") for d in range(2)]
                NG = EPC // 128
                for di, ed in enumerate((src_d, dst_d)):
                    ei = wp.tile([128, NG], I32, tag="ei")
                    dma(ei[:], ed[:])
                    qi = wp.tile([128, NG], I32, tag="qi")
                    ri = wp.tile([128, NG], I32, tag="ri")
                    nc.vector.tensor_scalar(out=qi[:], in0=ei[:], scalar1=7, scalar2=None,
                                            op0=ALU.logical_shift_right)
                    nc.vector.tensor_scalar(out=ri[:], in0=ei[:], scalar1=127, scalar2=None,
                                            op0=ALU.bitwise_and)
                    qf = wp.tile([128, NG], BF16, tag="qf")
                    rf = wp.tile([128, NG], BF16, tag="rf")
                    nc.vector.tensor_copy(qf[:], qi[:])
                    nc.vector.tensor_copy(rf[:], ri[:])
                    hps = pp.tile([16, 128], F32, space="PSUM", tag="psA")
                    for g in range(NG):
                        Bg = wp.tile([128, 128], BF16, tag="Bg")
                        Ag = wp.tile([128, 16], BF16, tag="Ag")
                        nc.vector.tensor_tensor(out=Bg[:], in0=iota128f[:],
                                                in1=rf[:, g:g + 1].to_broadcast([128, 128]),
                                                op=ALU.is_equal)
                        nc.vector.tensor_tensor(out=Ag[:], in0=iota16f[:],
                                                in1=qf[:, g:g + 1].to_broadcast([128, 16]),
                                                op=ALU.is_equal)
                        nc.tensor.matmul(out=hps[:], lhsT=Ag[:], rhs=Bg[:],
                                         start=(g == 0), stop=(g == NG - 1))
                    nc.vector.tensor_copy(cnts[di][:], hps[:])

                dma(ar_in[0:16, :], cnts[0][:])
                dma(ar_in[16:32, :], cnts[1][:])
                nc.gpsimd.collective_compute(
                    "AllReduce", ALU.add, replica_groups=[list(range(NC))],
                    ins=[ar_in.opt()], outs=[ar_out.opt()])

                # ---------- h0 + degree embeds (replicated) ----------
                for t in range(NT):
                    hb = pp.tile([128, D], F32, space="PSUM", tag="psA")
                    nc.tensor.matmul(out=hb[:], lhsT=xa[:, t * 128:(t + 1) * 128],
                                     rhs=wina[:], start=True, stop=False,
                                     skip_group_check=True)
                    for di, ztab in ((0, zout), (1, zin)):
                        crow = wp.tile([1, 128], F32, tag="crow")
                        dma(crow[:], ar_out[di * 16 + t:di * 16 + t + 1, :])
                        crow16 = wp.tile([1, 128], BF16, tag="crow16")
                        nc.vector.tensor_scalar(out=crow16[:], in0=crow[:],
                                                scalar1=float(MAXD - 1), scalar2=None,
                                                op0=ALU.min)
                        if DBG:
                            dma(dbg["cnt"][0:1, (di * 16 + t) * 128:(di * 16 + t + 1) * 128],
                                crow16[0:1, :])
                        bc = pp.tile([MAXD, 128], F32, space="PSUM", tag="psA")
                        nc.tensor.matmul(out=bc[:], lhsT=ones_bf[:1, :MAXD],
                                         rhs=crow16[0:1, :],
                                         start=True, stop=True)
                        oh = wp.tile([MAXD, 128], BF16, tag="oh")
                        nc.vector.tensor_scalar(out=oh[:], in0=bc[:], scalar1=io64[:, 0:1],
                                                scalar2=None, op0=ALU.is_equal)
                        nc.tensor.matmul(out=hb[:], lhsT=oh[:], rhs=ztab[:],
                                         start=False, stop=(di == 1),
                                         skip_group_check=True)
                    nc.vector.tensor_copy(hfull[t][:], hb[:])
                    dma(htmp_dram[t * 128:(t + 1) * 128, :], hfull[t][:])
                    if DBG:
                        dma(dbg["h0"][t * 128:(t + 1) * 128, :], hfull[t][:])
                for j in range(2):
                    nc.gpsimd.indirect_dma_start(
                        out=hown[j][:], out_offset=None, in_=htmp_dram[:],
                        in_offset=bass.IndirectOffsetOnAxis(ap=oidx[:, j:j + 1], axis=0))

            # ========================================================
            # LAYERS
            # ========================================================
            def ln_group(srcs, gcol, bcol, outs):
                """Batched LayerNorm: stats for all tiles, one Sqrt + one
                reciprocal, then per-tile normalize + transpose + g/b evict."""
                nT = len(srcs)
                var = wp.tile([128, NT + 2], F32, tag="lnvar")
                mean = wp.tile([128, NT + 2], F32, tag="lnmean")
                for i, s_ap in enumerate(srcs):
                    st6 = wp.tile([128, 6], F32, tag="st6")
                    nc.vector.bn_stats(out=st6[:], in_=s_ap)
                    mv = wp.tile([128, 2], F32, tag="mv")
                    nc.vector.bn_aggr(out=mv[:], in_=st6[:])
                    nc.vector.tensor_copy(mean[:, i:i + 1], mv[:, 0:1])
                    nc.vector.tensor_copy(var[:, i:i + 1], mv[:, 1:2])
                sd = wp.tile([128, NT + 2], F32, tag="lnsd")
                nc.scalar.activation(sd[:, :nT], var[:, :nT], AF.Sqrt, bias=eps5[:, 0:1])
                rstd = wp.tile([128, NT + 2], F32, tag="lnrstd")
                nc.vector.reciprocal(rstd[:, :nT], sd[:, :nT])
                for i, s_ap in enumerate(srcs):
                    xn = wp.tile([128, D], BF16, tag="xn")
                    nc.vector.tensor_scalar(out=xn[:], in0=s_ap, scalar1=mean[:, i:i + 1],
                                            scalar2=rstd[:, i:i + 1], op0=ALU.subtract,
                                            op1=ALU.mult)
                    trp = pp.tile([128, 128], BF16, space="PSUM", tag="psA")
                    nc.tensor.transpose(out=trp[:], in_=xn[:], identity=ident[:])
                    nc.scalar.activation(outs[i], trp[:], AF.Identity,
                                         bias=bcol, scale=gcol)

            for l in range(L):
                ln_group([hfull[t][:] for t in range(NT)] + [hown[j][:] for j in range(2)],
                         l1g[l], l1b[l],
                         [xnT[:, t * 128:(t + 1) * 128] for t in range(NT)]
                         + [xnTo[:, j * 128:(j + 1) * 128] for j in range(2)])
                if DBG and l == 0:
                    dma(dbg["xnT"][:], xnT[:])
                # V (all heads) per n-tile; evictions on GpSimd (idle engine)
                for t in range(NT):
                    for i in range(2):
                        vp = pp.tile([128, 512], F32, space="PSUM", tag="psA")
                        nc.tensor.matmul(out=vp[:],
                                         lhsT=xnT[:, t * 128:(t + 1) * 128],
                                         rhs=wv[l][:, i * 512:(i + 1) * 512],
                                         start=True, stop=True)
                        nc.scalar.activation(Vall[t][:, i * 512:(i + 1) * 512], vp[:],
                                             AF.Identity)
                # Q^T per head
                for h in range(H):
                    qp = pp.tile([D, RPC], F32, space="PSUM", tag="psA")
                    nc.tensor.matmul(out=qp[:], lhsT=wq[l][h], rhs=xnTo[:],
                                     start=True, stop=True)
                    nc.vector.tensor_scalar(out=QT[:, h * RPC:(h + 1) * RPC],
                                            in0=qp[:], scalar1=bq[l][h],
                                            scalar2=None, op0=ALU.add)
                # bo' = bo + sum_h bv_h @ Wo_h
                bop_ps = pp.tile([1, D], F32, space="PSUM", tag="psA")
                for h in range(H):
                    nc.tensor.matmul(out=bop_ps[:], lhsT=bv[l][h],
                                     rhs=wo[l][h], start=(h == 0), stop=(h == H - 1))
                bop = cp.tile([1, D], F32, name=f"bop{l}")
                nc.vector.tensor_tensor(out=bop[:], in0=bop_ps[:], in1=bo[l],
                                        op=ALU.add)
                # flash attention, two heads per pass
                hpt = [wp.tile([128, D], F32, name=f"hp{j}") for j in range(2)]
                accD = [wp.tile([128, D], F32, name=f"accD{j}") for j in range(2)]
                for hp2 in range(H // 2):
                    h0, h1 = 2 * hp2, 2 * hp2 + 1
                    KTa = wp.tile([D, N], BF16, tag="KTa", bufs=2)
                    KTb = wp.tile([D, N], BF16, tag="KTb", bufs=2)
                    for hh, KT in ((h0, KTa), (h1, KTb)):
                        for i in range(4):
                            kp = pp.tile([D, 512], F32, space="PSUM", tag="psA")
                            nc.tensor.matmul(out=kp[:], lhsT=wk[l][hh],
                                             rhs=xnT[:, i * 512:(i + 1) * 512],
                                             start=True, stop=True)
                            nc.vector.tensor_scalar(out=KT[:, i * 512:(i + 1) * 512],
                                                    in0=kp[:], scalar1=bk[l][hh],
                                                    scalar2=None, op0=ALU.add)
                    otp0 = pp.tile([D, RPC], F32, space="PSUM", tag="otp", bufs=1)
                    otp1 = pp.tile([D, RPC], F32, space="PSUM", tag="otp2", bufs=1)
                    rsp = pp.tile([1, 512], F32, space="PSUM", tag="rsp", bufs=1)
                    for kt in range(NT):
                        stp = pp.tile([128, 512], F32, space="PSUM", tag="stp", bufs=3)
                        nc.tensor.matmul(out=stp[:, 0:RPC],
                                         lhsT=KTa[:, kt * 128:(kt + 1) * 128],
                                         rhs=QT[:, h0 * RPC:(h0 + 1) * RPC],
                                         start=True, stop=True, skip_group_check=True)
                        nc.tensor.matmul(out=stp[:, RPC:2 * RPC],
                                         lhsT=KTb[:, kt * 128:(kt + 1) * 128],
                                         rhs=QT[:, h1 * RPC:(h1 + 1) * RPC],
                                         start=True, stop=True, skip_group_check=True)
                        PT = wp.tile([128, 512], BF16, tag="PT")
                        nc.scalar.activation(PT[:], stp[:], AF.Exp, scale=SCALE)
                        nc.vector.tensor_tensor(out=PT[:], in0=PT[:], in1=EB[kt][:],
                                                op=ALU.mult)
                        nc.tensor.matmul(out=rsp[:], lhsT=onesc_bf[:, 0:1], rhs=PT[:],
                                         start=(kt == 0), stop=(kt == NT - 1))
                        nc.tensor.matmul(out=otp0[:],
                                         lhsT=Vall[kt][:, h0 * D:(h0 + 1) * D],
                                         rhs=PT[:, 0:RPC],
                                         start=(kt == 0), stop=(kt == NT - 1))
                        nc.tensor.matmul(out=otp1[:],
                                         lhsT=Vall[kt][:, h1 * D:(h1 + 1) * D],
                                         rhs=PT[:, RPC:2 * RPC],
                                         start=(kt == 0), stop=(kt == NT - 1))
                        if DBG and l == 0 and hp2 == 0 and kt == 0:
                            dma(dbg["PT0"][:], PT[:, 0:RPC])
                    nc.vector.tensor_copy(OT[:, h0 * RPC:(h0 + 1) * RPC], otp0[:])
                    nc.vector.tensor_copy(OT[:, h1 * RPC:(h1 + 1) * RPC], otp1[:])
                    nc.vector.tensor_copy(rsr[:, hp2 * 512:(hp2 + 1) * 512], rsp[:])
                    if DBG and l == 0 and hp2 == 0:
                        dma(dbg["KT0"][:], KTa[:])
                        dma(dbg["QT0"][:], QT[:, 0:RPC])
                        dma(dbg["OT0"][:], OT[:, 0:RPC])
                        dma(dbg["rs0"][:], rsr[:, 0:RPC])
                rst = wp.tile([128, 2 * H], F32, tag="rst")
                for h in range(H):
                    for j in range(2):
                        rtp = pp.tile([128, 1], F32, space="PSUM", tag="psA")
                        nc.tensor.transpose(
                            out=rtp[:], in_=rsr[0:1, h * RPC + j * 128:h * RPC + (j + 1) * 128],
                            identity=identf[0:1, 0:1])
                        nc.vector.tensor_copy(rst[:, h * 2 + j:h * 2 + j + 1], rtp[:])
                nc.vector.reciprocal(rrc[:], rst[:])
                for j in range(2):
                    for h in range(H):
                        wdp = pp.tile([128, D], F32, space="PSUM", tag="psA")
                        if h == 0:
                            nc.tensor.matmul(out=wdp[:],
                                             lhsT=rsr[0:1, j * 128:(j + 1) * 128],
                                             rhs=bop[:], start=True, stop=False,
                                             skip_group_check=True)
                        nc.tensor.matmul(out=wdp[:],
                                         lhsT=OT[:, h * RPC + j * 128:h * RPC + (j + 1) * 128],
                                         rhs=wo[l][h], start=(h != 0), stop=True,
                                         skip_group_check=True)
                        if h == 0:
                            nc.vector.tensor_scalar(out=accD[j][:], in0=wdp[:],
                                                    scalar1=rrc[:, j:j + 1],
                                                    scalar2=None, op0=ALU.mult)
                        else:
                            tmpw = wp.tile([128, D], F32, tag="tmpw")
                            nc.vector.tensor_scalar(out=tmpw[:], in0=wdp[:],
                                                    scalar1=rrc[:, h * 2 + j:h * 2 + j + 1],
                                                    scalar2=None, op0=ALU.mult)
                            nc.vector.tensor_tensor(out=accD[j][:], in0=accD[j][:],
                                                    in1=tmpw[:], op=ALU.add)
                    nc.vector.tensor_tensor(out=hpt[j][:], in0=hown[j][:],
                                            in1=accD[j][:], op=ALU.add)
                # LN2 + FF on own rows
                ln_group([hpt[j][:] for j in range(2)], l2g[l], l2b[l],
                         [xn2T[:, j * 128:(j + 1) * 128] for j in range(2)])
                for fs in range(4):
                    fp = pp.tile([128, RPC], F32, space="PSUM", tag="psA")
                    nc.tensor.matmul(out=fp[:], lhsT=w1[l][:, fs * 128:(fs + 1) * 128],
                                     rhs=xn2T[:], start=True, stop=True)
                    nc.scalar.activation(gT[fs][:], fp[:],
                                         AF.Gelu, bias=b1c[l][fs])
                fdp = pp.tile([D, RPC], F32, space="PSUM", tag="psA")
                for fs in range(4):
                    nc.tensor.matmul(out=fdp[:], lhsT=w2[l][fs],
                                     rhs=gT[fs][:],
                                     start=(fs == 0), stop=(fs == 3))
                ffdT = wp.tile([D, RPC], BF16, tag="ffdT")
                nc.scalar.activation(ffdT[:], fdp[:], AF.Identity, bias=b2c[l])
                for j in range(2):
                    ftp = pp.tile([128, 128], BF16, space="PSUM", tag="psA")
                    nc.tensor.transpose(out=ftp[:], in_=ffdT[:, j * 128:(j + 1) * 128],
                                        identity=ident[:])
                    nc.vector.tensor_tensor(out=hown[j][:], in0=hpt[j][:], in1=ftp[:],
                                            op=ALU.add)
                if DBG and l == 0:
                    for j in range(2):
                        dma(dbg["hL0"][j * 128:(j + 1) * 128, :], hown[j][:])
                # AllGather residual (not needed after last layer)
                if l < L - 1:
                    for j in range(2):
                        dma(ag_in[l][j * 128:(j + 1) * 128, :], hown[j][:])
                    nc.gpsimd.collective_compute(
                        "AllGather", ALU.bypass, replica_groups=[list(range(NC))],
                        ins=[ag_in[l].opt()], outs=[ag_out[l].opt()])
                    for t in range(NT):
                        dma(hfull[t][:], ag_out[l][t * 128:(t + 1) * 128, :])

            # ================= output =================
            for j in range(2):
                hb16 = wp.tile([128, D], BF16, tag="hb16")
                nc.vector.tensor_copy(hb16[:], hown[j][:])
                htp = pp.tile([128, 128], BF16, space="PSUM", tag="psA")
                nc.tensor.transpose(out=htp[:], in_=hb16[:], identity=ident[:])
                hT = wp.tile([D, 128], BF16, tag="hT")
                nc.vector.tensor_copy(hT[:], htp[:])
                op_ps = pp.tile([OUT, 128], F32, space="PSUM", tag="psA")
                nc.tensor.matmul(out=op_ps[:], lhsT=ow, rhs=hT[:],
                                 start=True, stop=True)
                ob_sb = wp.tile([OUT, 128], F32, tag="ob_sb")
                nc.scalar.activation(ob_sb[:], op_ps[:], AF.Identity,
                                     bias=obc[:, 0:1])
                dma(out_d[:, j * 128:(j + 1) * 128], ob_sb[:])

    nc.finalize()
    return nc


def _pack_bf16(I):
    Wq, Wk, Wo_, Wv_ = f32(I["Wq"]), f32(I["Wk"]), f32(I["Wo"]), f32(I["Wv"])
    cols = []
    for l in range(L):
        for h in range(H):
            cols.append(Wq[l, h])
    for l in range(L):
        for h in range(H):
            cols.append(Wk[l, h])
    for l in range(L):
        for h in range(H):
            cols.append(Wo_[l, h * D:(h + 1) * D, :])
    for l in range(L):
        cols.append(Wv_[l].transpose(1, 0, 2).reshape(D, H * D))
    for l in range(L):
        cols.append(f32(I["ff1_w"])[l])
    for l in range(L):
        for fs in range(4):
            cols.append(f32(I["ff2_w"])[l, fs * 128:(fs + 1) * 128, :])
    cols.append(f32(I["out_w"]))
    bvv = f32(I["bv"])
    for l in range(L):
        for h in range(H):
            cols.append(bvv[l, h][:, None])
    out = np.concatenate(cols, 1)
    assert out.shape == (128, NBF), out.shape
    return bf16(out)


def _pack_f32(I):
    cols = []
    for l in range(L):
        for h in range(H):
            cols.append(f32(I["bq"])[l, h][:, None])
    for l in range(L):
        for h in range(H):
            cols.append(f32(I["bk"])[l, h][:, None])
    for nm in ("ln1_g", "ln1_b", "ln2_g", "ln2_b"):
        for l in range(L):
            cols.append(f32(I[nm])[l][:, None])
    for l in range(L):
        cols.append(f32(I["ff2_b"])[l][:, None])
    for l in range(L):
        for fs in range(4):
            cols.append(f32(I["ff1_b"])[l, fs * 128:(fs + 1) * 128][:, None])
    out = np.concatenate(cols, 1)
    assert out.shape == (128, NF32), out.shape
    return f32(out)


def _prep(inputs):
    I = {k: np.asarray(v) for k, v in inputs.items()}
    x = f32(I["x"])
    pos = f32(I["pos"])
    ei = I["edge_index"].astype(np.int32)
    Wv = f32(I["Wv"])

    common = {
        "xa": bf16(np.concatenate([x.T, np.ones((1, N), np.float32)], 0)),
        "wina": bf16(np.concatenate([f32(I["node_in_w"]), f32(I["node_in_b"])[None]], 0)),
        "zin": bf16(I["z_in"]),
        "zout": bf16(I["z_out"]),
        "posT": bf16(pos.T),
        "wb": _pack_bf16(I),
        "pf": _pack_f32(I),
        "boall": f32(I["bo"]).reshape(1, L * D),
        "obc": f32(I["out_b"])[:, None],
        "spr": np.concatenate([f32(I["sp_mu"]), f32(I["sp_sigma"]), f32(I["sp_w"]),
                               np.full(HS, float(I["sp_b"]), np.float32)])[None].astype(np.float32),
        "iota64": np.arange(MAXD, dtype=np.float32)[:, None],
    }
    common = {k: np.ascontiguousarray(v) for k, v in common.items()}
    in_maps = []
    for c in range(NC):
        m = dict(common)
        qsl = slice(c * RPC, (c + 1) * RPC)
        m["pcT"] = bf16(pos[qsl].T)
        m["src"] = np.ascontiguousarray(
            ei[0, c * EPC:(c + 1) * EPC].reshape(EPC // 128, 128).T)
        m["dst"] = np.ascontiguousarray(
            ei[1, c * EPC:(c + 1) * EPC].reshape(EPC // 128, 128).T)
        m["oidx"] = np.ascontiguousarray(
            (c * RPC + np.arange(RPC, dtype=np.int32)).reshape(2, 128).T)
        in_maps.append(m)
    return in_maps


def kernel(**inputs) -> np.ndarray:
    if "nc" not in _cached:
        _cached["nc"] = build()
    in_maps = _prep(inputs)
    res = run_bass_kernel_spmd(_cached["nc"], in_maps, core_ids=list(range(NC)))
    _cached["last_results"] = res
    out = np.concatenate([f32(r["out"]).T for r in res.results], 0)
    return out.astype(np.float32)


# revision 23
# speedup vs baseline: 1.1574x; 1.0472x over previous
"""DynaFormer (Graphormer-style GNN transformer) on 8 TRN2 NeuronCores.

Sharding: sequence-parallel over the N=2048 query rows (256/core).
Params replicated; K/V computed replicated from an all-gathered residual;
attention (scores/softmax/PV), Wo, LN2, FF row-sharded. Collectives:
1 AllReduce (degree histograms, 16KB) + 2 AllGathers (residual, 1MB).

Self-contained: hardcodes all shapes; builds + compiles + runs the Bass
kernel via run_bass_kernel_spmd on cores 0-7.
"""
import os
import numpy as np
import ml_dtypes

import concourse.bass as bass
import concourse.bacc as bacc
import concourse.tile as tile
import concourse.mybir as mybir
from concourse.bass_utils import run_bass_kernel_spmd
from concourse.masks import make_identity

dt = mybir.dt
F32 = dt.float32
BF16 = dt.bfloat16
I32 = dt.int32
AF = mybir.ActivationFunctionType
ALU = mybir.AluOpType

N, E, IN_NODE, D = 2048, 65536, 16, 128
H, FFD, HS, L = 8, 512, 8, 3
MAXD, OUT = 64, 64
NC = 8
RPC = N // NC            # 256 rows/core
EPC = E // NC            # 8192 edges/core
NT = N // 128            # 16 node tiles
SCALE = float(1.0 / np.sqrt(D))

# packed weight segment layout: name -> (count, width)
_BSEGS = [("wq", 24, 128), ("wk", 24, 128), ("wo", 24, 128), ("wv", 3, 1024),
          ("w1", 3, 512), ("w2", 12, 128), ("ow", 1, 64), ("bv", 24, 1)]
_FSEGS = [("bq", 24), ("bk", 24), ("l1g", 3), ("l1b", 3), ("l2g", 3), ("l2b", 3),
          ("b2c", 3), ("b1c", 12)]
BOFF = {}
_o = 0
for _n, _c, _w in _BSEGS:
    BOFF[_n] = _o
    _o += _c * _w
NBF = _o
FOFF = {}
_o = 0
for _n, _c in _FSEGS:
    FOFF[_n] = _o
    _o += _c
NF32 = _o

DBG = bool(int(os.environ.get("BASS_DBG", "0")))

_cached = {}


def bf16(x):
    return np.ascontiguousarray(np.asarray(x, np.float32).astype(ml_dtypes.bfloat16))


def f32(x):
    return np.ascontiguousarray(np.asarray(x, np.float32))


def build():
    nc = bacc.Bacc("TRN2", target_bir_lowering=False, debug=False,
                   enable_asserts=True, num_devices=NC)

    def din(name, shape, dty):
        return nc.dram_tensor(name, list(shape), dty, kind="ExternalInput").ap()

    def dout(name, shape, dty=F32):
        return nc.dram_tensor(name, list(shape), dty, kind="ExternalOutput").ap()

    # ---- dram params (common)
    xa_d = din("xa", [IN_NODE + 1, N], BF16)
    zin_d = din("zin", [MAXD, D], BF16)
    zout_d = din("zout", [MAXD, D], BF16)
    posT_d = din("posT", [3, N], BF16)
    wb_d = din("wb", [128, NBF], BF16)
    pf_d = din("pf", [128, NF32], F32)
    boall_d = din("boall", [1, L * D], F32)
    wina_d = din("wina", [IN_NODE + 1, D], BF16)
    obc_d = din("obc", [OUT, 1], F32)
    spr_d = din("spr", [1, 4 * HS], F32)   # mu | sigma | w | sp_b*8
    io64_d = din("iota64", [MAXD, 1], F32)
    # ---- per-core
    pcT_d = din("pcT", [3, RPC], BF16)
    src_d = din("src", [128, EPC // 128], I32)
    dst_d = din("dst", [128, EPC // 128], I32)
    oidx_d = din("oidx", [128, 2], I32)
    out_d = dout("out", [OUT, RPC], F32)
    dbg = {}
    if DBG:
        dbg["h0"] = dout("dbg_h0", [N, D])
        dbg["EB0"] = dout("dbg_EB0", [128, 512], BF16)
        dbg["xnT"] = dout("dbg_xnT", [D, N], BF16)
        dbg["KT0"] = dout("dbg_KT0", [D, N], BF16)
        dbg["QT0"] = dout("dbg_QT0", [D, RPC], BF16)
        dbg["PT0"] = dout("dbg_PT0", [128, RPC], BF16)
        dbg["OT0"] = dout("dbg_OT0", [D, RPC], BF16)
        dbg["rs0"] = dout("dbg_rs0", [1, RPC])
        dbg["hL0"] = dout("dbg_hL0", [RPC, D])
        dbg["cnt"] = dout("dbg_cnt", [1, 4096], BF16)

    with tile.TileContext(nc) as tc:
        with tc.tile_pool(name="cp", bufs=1) as cp, \
             tc.tile_pool(name="wp", bufs=3) as wp, \
             tc.tile_pool(name="pp", bufs=2, space="PSUM") as pp, \
             tc.tile_pool(name="dp", bufs=1, space="DRAM") as dp:

            dma = nc.sync.dma_start

            # ---------- persistent constants ----------
            ident = cp.tile([128, 128], BF16)
            make_identity(nc, ident[:])
            identf = cp.tile([128, 128], F32)
            make_identity(nc, identf[:])
            onesc_bf = cp.tile([128, 1], BF16)
            nc.vector.memset(onesc_bf[:], 1.0)
            eps5 = cp.tile([128, 1], F32)
            nc.vector.memset(eps5[:], 1e-5)

            WB = cp.tile([128, NBF], BF16)
            dma(WB[:], wb_d[:])
            PF = cp.tile([128, NF32], F32)
            dma(PF[:], pf_d[:])
            bo_all = cp.tile([1, L * D], F32)
            dma(bo_all[:], boall_d[:])
            obc = cp.tile([OUT, 1], F32)
            dma(obc[:], obc_d[:])

            def bseg(name, i, w):
                off = BOFF[name] + i * w
                return WB[:, off:off + w]

            def fseg(name, i):
                off = FOFF[name] + i
                return PF[:, off:off + 1]

            wq = [[bseg("wq", l * H + h, D) for h in range(H)] for l in range(L)]
            wk = [[bseg("wk", l * H + h, D) for h in range(H)] for l in range(L)]
            wo = [[bseg("wo", l * H + h, D) for h in range(H)] for l in range(L)]
            wv = [bseg("wv", l, H * D) for l in range(L)]
            w1 = [bseg("w1", l, FFD) for l in range(L)]
            w2 = [[bseg("w2", l * 4 + fs, D) for fs in range(4)] for l in range(L)]
            ow = bseg("ow", 0, OUT)
            bv = [[bseg("bv", l * H + h, 1) for h in range(H)] for l in range(L)]
            bq = [[fseg("bq", l * H + h) for h in range(H)] for l in range(L)]
            bk = [[fseg("bk", l * H + h) for h in range(H)] for l in range(L)]
            l1g = [fseg("l1g", l) for l in range(L)]
            l1b = [fseg("l1b", l) for l in range(L)]
            l2g = [fseg("l2g", l) for l in range(L)]
            l2b = [fseg("l2b", l) for l in range(L)]
            b2c = [fseg("b2c", l) for l in range(L)]
            b1c = [[fseg("b1c", l * 4 + fs) for fs in range(4)] for l in range(L)]
            bo = [bo_all[0:1, l * D:(l + 1) * D] for l in range(L)]

            # persistent activations
            hfull = [cp.tile([128, D], F32, name=f"hf{t}") for t in range(NT)]
            hown = [cp.tile([128, D], F32, name=f"ho{j}") for j in range(2)]
            EB = [cp.tile([128, 512], BF16, name=f"EB{kt}") for kt in range(NT)]
            xnT = cp.tile([D, N], BF16)
            xnTo = cp.tile([D, RPC], BF16)
            Vall = [cp.tile([128, H * D], BF16, name=f"V{t}") for t in range(NT)]
            QT = cp.tile([D, H * RPC], BF16)
            OT = cp.tile([D, H * RPC], BF16)
            rsr = cp.tile([1, H * RPC], F32)
            rrc = cp.tile([128, 2 * H], F32)
            xn2T = cp.tile([D, RPC], BF16)
            gT = [cp.tile([128, RPC], BF16, name=f"gT{fs}") for fs in range(4)]
            htmp_dram = dp.tile([N, D], F32)
            ar_in = dp.tile([32, 128], F32)
            ar_out = dp.tile([32, 128], F32)
            ag_in = [dp.tile([RPC, D], F32, name=f"agi{l}") for l in range(L - 1)]
            ag_out = [dp.tile([N, D], F32, name=f"ago{l}") for l in range(L - 1)]

            # ========================================================
            # PREPROC (scoped pool: tiles freed before the layer loop)
            # ========================================================
            with tc.tile_pool(name="prep", bufs=1) as prp:
                xa = prp.tile([IN_NODE + 1, N], BF16)
                dma(xa[:], xa_d[:])
                wina = prp.tile([IN_NODE + 1, D], BF16)
                dma(wina[:], wina_d[:])
                zin = prp.tile([MAXD, D], BF16)
                dma(zin[:], zin_d[:])
                zout = prp.tile([MAXD, D], BF16)
                dma(zout[:], zout_d[:])
                posT = prp.tile([3, N], BF16)
                dma(posT[:], posT_d[:])
                pcT = prp.tile([3, RPC], BF16)
                dma(pcT[:], pcT_d[:])
                io64 = prp.tile([MAXD, 1], F32)
                dma(io64[:], io64_d[:])
                spr = prp.tile([1, 4 * HS], F32)
                dma(spr[:], spr_d[:])
                oidx = prp.tile([128, 2], I32)
                dma(oidx[:], oidx_d[:])
                ones_bf = prp.tile([1, 128], BF16)
                nc.vector.memset(ones_bf[:], 1.0)
                ones_f = prp.tile([1, 128], F32)
                nc.vector.memset(ones_f[:], 1.0)
                ones3_f = prp.tile([3, 1], F32)
                nc.vector.memset(ones3_f[:], 1.0)
                eps12 = prp.tile([128, 1], F32)
                nc.vector.memset(eps12[:], 1e-12)

                # ---------- spatial bias -> EB = exp(b) duplicated [128, 512] ----------
                p5 = prp.tile([5, N], BF16)
                nc.vector.tensor_scalar(out=p5[0:3, :], in0=posT[:], scalar1=-2.0,
                                        scalar2=None, op0=ALU.mult)
                onesN = prp.tile([1, 512], BF16)
                nc.vector.memset(onesN[:], 1.0)
                for i in range(4):
                    dma(p5[4:5, i * 512:(i + 1) * 512], onesN[0:1, :])
                for i in range(4):
                    sq3 = wp.tile([3, 512], F32, tag="sq3", bufs=2)
                    nc.scalar.activation(sq3[:], posT[:, i * 512:(i + 1) * 512], AF.Square)
                    njp = pp.tile([1, 512], F32, space="PSUM", tag="psA")
                    nc.tensor.matmul(out=njp[:], lhsT=ones3_f[:3, 0:1],
                                     rhs=sq3[:], start=True, stop=True)
                    njrow = wp.tile([1, 512], BF16, tag="njrow", bufs=2)
                    nc.vector.tensor_copy(njrow[:], njp[:])
                    dma(p5[3:4, i * 512:(i + 1) * 512], njrow[0:1, :])
                r5 = prp.tile([5, RPC], BF16)
                nc.vector.tensor_copy(r5[0:3, :], pcT[:])
                dma(r5[3:4, :], onesN[0:1, 0:RPC])
                sqc = wp.tile([3, RPC], F32, tag="sqc", bufs=1)
                nc.scalar.activation(sqc[:], pcT[:], AF.Square)
                nqp = pp.tile([1, RPC], F32, space="PSUM", tag="psA")
                nc.tensor.matmul(out=nqp[:], lhsT=ones3_f[:3, 0:1], rhs=sqc[:],
                                 start=True, stop=True)
                nqrow = wp.tile([1, RPC], BF16, tag="nqrow", bufs=1)
                nc.vector.tensor_copy(nqrow[:], nqp[:])
                dma(r5[4:5, :], nqrow[0:1, :])

                # sp params broadcast to [128, HS] via K=1 matmuls
                spc = []
                for i in range(4):
                    sps = pp.tile([128, HS], F32, space="PSUM", tag="psA")
                    nc.tensor.matmul(out=sps[:], lhsT=ones_f[:1, :], rhs=spr[0:1, i * 8:i * 8 + 8],
                                     start=True, stop=True, skip_group_check=True)
                    t_ = prp.tile([128, HS], F32, name=f"spc{i}")
                    nc.vector.tensor_copy(t_[:], sps[:])
                    spc.append(t_)
                mu_bc, sg_bc, w_bc, b0_bc = spc
                rsig = prp.tile([128, HS], F32)
                nc.vector.reciprocal(rsig[:], sg_bc[:])
                msc = prp.tile([128, HS], F32)
                nc.vector.tensor_tensor(out=msc[:], in0=mu_bc[:], in1=rsig[:], op=ALU.mult)
                nc.vector.tensor_scalar(out=msc[:], in0=msc[:], scalar1=-1.0, scalar2=None,
                                        op0=ALU.mult)

                # all d-tiles first (one Sqrt table), then gaussians (Square/Exp set)
                dds = [prp.tile([128, RPC], BF16, name=f"dd{kt}") for kt in range(NT)]
                for kt in range(NT):
                    d2p = pp.tile([128, RPC], F32, space="PSUM", tag="psA")
                    nc.tensor.matmul(out=d2p[:], lhsT=p5[:, kt * 128:(kt + 1) * 128],
                                     rhs=r5[:], start=True, stop=True)
                    d2c = wp.tile([128, RPC], F32, tag="d2c")
                    nc.vector.tensor_scalar(out=d2c[:], in0=d2p[:], scalar1=0.0,
                                            scalar2=None, op0=ALU.max)
                    nc.scalar.activation(dds[kt][:], d2c[:], AF.Sqrt, bias=eps12[:, 0:1])
                for kt in range(NT):
                    bacc_t = wp.tile([128, RPC], BF16, tag="bacc")
                    for s in range(HS):
                        t1 = wp.tile([128, RPC], F32, tag="t1")
                        nc.scalar.activation(t1[:], dds[kt][:], AF.Square,
                                             bias=msc[:, s:s + 1], scale=rsig[:, s:s + 1])
                        g = wp.tile([128, RPC], BF16, tag="gga")
                        nc.scalar.activation(g[:], t1[:], AF.Exp, scale=-0.5)
                        if s == 0:
                            nc.vector.tensor_scalar(out=bacc_t[:], in0=g[:],
                                                    scalar1=w_bc[:, 0:1], scalar2=b0_bc[:, 0:1],
                                                    op0=ALU.mult, op1=ALU.add)
                        else:
                            gs = wp.tile([128, RPC], BF16, tag="gs")
                            nc.vector.tensor_scalar(out=gs[:], in0=g[:],
                                                    scalar1=w_bc[:, s:s + 1], scalar2=None,
                                                    op0=ALU.mult)
                            nc.vector.tensor_tensor(out=bacc_t[:], in0=bacc_t[:], in1=gs[:],
                                                    op=ALU.add)
                    nc.scalar.activation(EB[kt][:, 0:RPC], bacc_t[:], AF.Exp)
                    nc.vector.tensor_copy(EB[kt][:, RPC:2 * RPC], EB[kt][:, 0:RPC])
                    if DBG and kt == 0:
                        dma(dbg["EB0"][:], EB[0][:])

                # ---------- degree histograms ----------
                iota128 = prp.tile([128, 128], I32)
                nc.gpsimd.iota(iota128[:], pattern=[[1, 128]], base=0, channel_multiplier=0)
                iota128f = prp.tile([128, 128], BF16)
                nc.vector.tensor_copy(iota128f[:], iota128[:])
                iota16 = prp.tile([128, 16], I32)
                nc.gpsimd.iota(iota16[:], pattern=[[1, 16]], base=0, channel_multiplier=0)
                iota16f = prp.tile([128, 16], BF16)
                nc.vector.tensor_copy(iota16f[:], iota16[:])

                cnts = [prp.tile([16, 128], F32, name=f"cnts{d}") for d in range(2)]
                NG = EPC // 128
                for di, ed in enumerate((src_d, dst_d)):
                    ei = wp.tile([128, NG], I32, tag="ei")
                    dma(ei[:], ed[:])
                    qi = wp.tile([128, NG], I32, tag="qi")
                    ri = wp.tile([128, NG], I32, tag="ri")
                    nc.vector.tensor_scalar(out=qi[:], in0=ei[:], scalar1=7, scalar2=None,
                                            op0=ALU.logical_shift_right)
                    nc.vector.tensor_scalar(out=ri[:], in0=ei[:], scalar1=127, scalar2=None,
                                            op0=ALU.bitwise_and)
                    qf = wp.tile([128, NG], BF16, tag="qf")
                    rf = wp.tile([128, NG], BF16, tag="rf")
                    nc.vector.tensor_copy(qf[:], qi[:])
                    nc.vector.tensor_copy(rf[:], ri[:])
                    hps = pp.tile([16, 128], F32, space="PSUM", tag="psA")
                    for g in range(NG):
                        Bg = wp.tile([128, 128], BF16, tag="Bg")
                        Ag = wp.tile([128, 16], BF16, tag="Ag")
                        nc.vector.tensor_tensor(out=Bg[:], in0=iota128f[:],
                                                in1=rf[:, g:g + 1].to_broadcast([128, 128]),
                                                op=ALU.is_equal)
                        nc.vector.tensor_tensor(out=Ag[:], in0=iota16f[:],
                                                in1=qf[:, g:g + 1].to_broadcast([128, 16]),
                                                op=ALU.is_equal)
                        nc.tensor.matmul(out=hps[:], lhsT=Ag[:], rhs=Bg[:],
                                         start=(g == 0), stop=(g == NG - 1))
                    nc.vector.tensor_copy(cnts[di][:], hps[:])

                dma(ar_in[0:16, :], cnts[0][:])
                dma(ar_in[16:32, :], cnts[1][:])
                nc.gpsimd.collective_compute(
                    "AllReduce", ALU.add, replica_groups=[list(range(NC))],
                    ins=[ar_in.opt()], outs=[ar_out.opt()])

                # ---------- h0 + degree embeds (replicated) ----------
                for t in range(NT):
                    hb = pp.tile([128, D], F32, space="PSUM", tag="psA")
                    nc.tensor.matmul(out=hb[:], lhsT=xa[:, t * 128:(t + 1) * 128],
                                     rhs=wina[:], start=True, stop=False,
                                     skip_group_check=True)
                    for di, ztab in ((0, zout), (1, zin)):
                        crow = wp.tile([1, 128], F32, tag="crow")
                        dma(crow[:], ar_out[di * 16 + t:di * 16 + t + 1, :])
                        crow16 = wp.tile([1, 128], BF16, tag="crow16")
                        nc.vector.tensor_scalar(out=crow16[:], in0=crow[:],
                                                scalar1=float(MAXD - 1), scalar2=None,
                                                op0=ALU.min)
                        if DBG:
                            dma(dbg["cnt"][0:1, (di * 16 + t) * 128:(di * 16 + t + 1) * 128],
                                crow16[0:1, :])
                        bc = pp.tile([MAXD, 128], F32, space="PSUM", tag="psA")
                        nc.tensor.matmul(out=bc[:], lhsT=ones_bf[:1, :MAXD],
                                         rhs=crow16[0:1, :],
                                         start=True, stop=True)
                        oh = wp.tile([MAXD, 128], BF16, tag="oh")
                        nc.vector.tensor_scalar(out=oh[:], in0=bc[:], scalar1=io64[:, 0:1],
                                                scalar2=None, op0=ALU.is_equal)
                        nc.tensor.matmul(out=hb[:], lhsT=oh[:], rhs=ztab[:],
                                         start=False, stop=(di == 1),
                                         skip_group_check=True)
                    nc.vector.tensor_copy(hfull[t][:], hb[:])
                    dma(htmp_dram[t * 128:(t + 1) * 128, :], hfull[t][:])
                    if DBG:
                        dma(dbg["h0"][t * 128:(t + 1) * 128, :], hfull[t][:])
                for j in range(2):
                    nc.gpsimd.indirect_dma_start(
                        out=hown[j][:], out_offset=None, in_=htmp_dram[:],
                        in_offset=bass.IndirectOffsetOnAxis(ap=oidx[:, j:j + 1], axis=0))

            # ========================================================
            # LAYERS
            # ========================================================
            def ln_group(srcs, gcol, bcol, outs):
                """Batched LayerNorm: stats for all tiles, one Sqrt + one
                reciprocal, then per-tile normalize + transpose + g/b evict."""
                nT = len(srcs)
                var = wp.tile([128, NT + 2], F32, tag="lnvar")
                mean = wp.tile([128, NT + 2], F32, tag="lnmean")
                for i, s_ap in enumerate(srcs):
                    st6 = wp.tile([128, 6], F32, tag="st6")
                    nc.vector.bn_stats(out=st6[:], in_=s_ap)
                    mv = wp.tile([128, 2], F32, tag="mv")
                    nc.vector.bn_aggr(out=mv[:], in_=st6[:])
                    nc.vector.tensor_copy(mean[:, i:i + 1], mv[:, 0:1])
                    nc.vector.tensor_copy(var[:, i:i + 1], mv[:, 1:2])
                sd = wp.tile([128, NT + 2], F32, tag="lnsd")
                nc.scalar.activation(sd[:, :nT], var[:, :nT], AF.Sqrt, bias=eps5[:, 0:1])
                rstd = wp.tile([128, NT + 2], F32, tag="lnrstd")
                nc.vector.reciprocal(rstd[:, :nT], sd[:, :nT])
                for i, s_ap in enumerate(srcs):
                    xn = wp.tile([128, D], BF16, tag="xn")
                    nc.vector.tensor_scalar(out=xn[:], in0=s_ap, scalar1=mean[:, i:i + 1],
                                            scalar2=rstd[:, i:i + 1], op0=ALU.subtract,
                                            op1=ALU.mult)
                    trp = pp.tile([128, 128], BF16, space="PSUM", tag="psA")
                    nc.tensor.transpose(out=trp[:], in_=xn[:], identity=ident[:])
                    nc.scalar.activation(outs[i], trp[:], AF.Identity,
                                         bias=bcol, scale=gcol)

            for l in range(L):
                ln_group([hfull[t][:] for t in range(NT)] + [hown[j][:] for j in range(2)],
                         l1g[l], l1b[l],
                         [xnT[:, t * 128:(t + 1) * 128] for t in range(NT)]
                         + [xnTo[:, j * 128:(j + 1) * 128] for j in range(2)])
                if DBG and l == 0:
                    dma(dbg["xnT"][:], xnT[:])
                # V (all heads) per n-tile; evictions on GpSimd (idle engine)
                for t in range(NT):
                    for i in range(2):
                        vp = pp.tile([128, 512], F32, space="PSUM", tag="psA")
                        nc.tensor.matmul(out=vp[:],
                                         lhsT=xnT[:, t * 128:(t + 1) * 128],
                                         rhs=wv[l][:, i * 512:(i + 1) * 512],
                                         start=True, stop=True)
                        nc.scalar.activation(Vall[t][:, i * 512:(i + 1) * 512], vp[:],
                                             AF.Identity)
                # Q^T per head
                for h in range(H):
                    qp = pp.tile([D, RPC], F32, space="PSUM", tag="psA")
                    nc.tensor.matmul(out=qp[:], lhsT=wq[l][h], rhs=xnTo[:],
                                     start=True, stop=True)
                    nc.vector.tensor_scalar(out=QT[:, h * RPC:(h + 1) * RPC],
                                            in0=qp[:], scalar1=bq[l][h],
                                            scalar2=None, op0=ALU.add)
                # bo' = bo + sum_h bv_h @ Wo_h
                bop_ps = pp.tile([1, D], F32, space="PSUM", tag="psA")
                for h in range(H):
                    nc.tensor.matmul(out=bop_ps[:], lhsT=bv[l][h],
                                     rhs=wo[l][h], start=(h == 0), stop=(h == H - 1))
                bop = cp.tile([1, D], F32, name=f"bop{l}")
                nc.vector.tensor_tensor(out=bop[:], in0=bop_ps[:], in1=bo[l],
                                        op=ALU.add)
                # flash attention, two heads per pass
                hpt = [wp.tile([128, D], F32, name=f"hp{j}") for j in range(2)]
                accD = [wp.tile([128, D], F32, name=f"accD{j}") for j in range(2)]
                for hp2 in range(H // 2):
                    h0, h1 = 2 * hp2, 2 * hp2 + 1
                    KTa = wp.tile([D, N], BF16, tag="KTa", bufs=2)
                    KTb = wp.tile([D, N], BF16, tag="KTb", bufs=2)
                    for hh, KT in ((h0, KTa), (h1, KTb)):
                        for i in range(4):
                            kp = pp.tile([D, 512], F32, space="PSUM", tag="psA")
                            nc.tensor.matmul(out=kp[:], lhsT=wk[l][hh],
                                             rhs=xnT[:, i * 512:(i + 1) * 512],
                                             start=True, stop=True)
                            nc.vector.tensor_scalar(out=KT[:, i * 512:(i + 1) * 512],
                                                    in0=kp[:], scalar1=bk[l][hh],
                                                    scalar2=None, op0=ALU.add)
                    otp0 = pp.tile([D, RPC], F32, space="PSUM", tag="otp", bufs=1)
                    otp1 = pp.tile([D, RPC], F32, space="PSUM", tag="otp2", bufs=1)
                    rsp = pp.tile([1, 512], F32, space="PSUM", tag="rsp", bufs=1)
                    for kt in range(NT):
                        stp = pp.tile([128, 512], F32, space="PSUM", tag="stp", bufs=3)
                        nc.tensor.matmul(out=stp[:, 0:RPC],
                                         lhsT=KTa[:, kt * 128:(kt + 1) * 128],
                                         rhs=QT[:, h0 * RPC:(h0 + 1) * RPC],
                                         start=True, stop=True, skip_group_check=True)
                        nc.tensor.matmul(out=stp[:, RPC:2 * RPC],
                                         lhsT=KTb[:, kt * 128:(kt + 1) * 128],
                                         rhs=QT[:, h1 * RPC:(h1 + 1) * RPC],
                                         start=True, stop=True, skip_group_check=True)
                        PT = wp.tile([128, 512], BF16, tag="PT")
                        nc.scalar.activation(PT[:], stp[:], AF.Exp, scale=SCALE)
                        nc.vector.tensor_tensor(out=PT[:], in0=PT[:], in1=EB[kt][:],
                                                op=ALU.mult)
                        nc.tensor.matmul(out=rsp[:], lhsT=onesc_bf[:, 0:1], rhs=PT[:],
                                         start=(kt == 0), stop=(kt == NT - 1))
                        nc.tensor.matmul(out=otp0[:],
                                         lhsT=Vall[kt][:, h0 * D:(h0 + 1) * D],
                                         rhs=PT[:, 0:RPC],
                                         start=(kt == 0), stop=(kt == NT - 1))
                        nc.tensor.matmul(out=otp1[:],
                                         lhsT=Vall[kt][:, h1 * D:(h1 + 1) * D],
                                         rhs=PT[:, RPC:2 * RPC],
                                         start=(kt == 0), stop=(kt == NT - 1))
                        if DBG and l == 0 and hp2 == 0 and kt == 0:
                            dma(dbg["PT0"][:], PT[:, 0:RPC])
                    nc.vector.tensor_copy(OT[:, h0 * RPC:(h0 + 1) * RPC], otp0[:])
                    nc.vector.tensor_copy(OT[:, h1 * RPC:(h1 + 1) * RPC], otp1[:])
                    nc.vector.tensor_copy(rsr[:, hp2 * 512:(hp2 + 1) * 512], rsp[:])
                    if DBG and l == 0 and hp2 == 0:
                        dma(dbg["KT0"][:], KTa[:])
                        dma(dbg["QT0"][:], QT[:, 0:RPC])
                        dma(dbg["OT0"][:], OT[:, 0:RPC])
                        dma(dbg["rs0"][:], rsr[:, 0:RPC])
                rst = wp.tile([128, 2 * H], F32, tag="rst")
                for h in range(H):
                    for j in range(2):
                        rtp = pp.tile([128, 1], F32, space="PSUM", tag="psA")
                        nc.tensor.transpose(
                            out=rtp[:], in_=rsr[0:1, h * RPC + j * 128:h * RPC + (j + 1) * 128],
                            identity=identf[0:1, 0:1])
                        nc.vector.tensor_copy(rst[:, h * 2 + j:h * 2 + j + 1], rtp[:])
                nc.vector.reciprocal(rrc[:], rst[:])
                for j in range(2):
                    for h in range(H):
                        wdp = pp.tile([128, D], F32, space="PSUM", tag="psA")
                        if h == 0:
                            nc.tensor.matmul(out=wdp[:],
                                             lhsT=rsr[0:1, j * 128:(j + 1) * 128],
                                             rhs=bop[:], start=True, stop=False,
                                             skip_group_check=True)
                        nc.tensor.matmul(out=wdp[:],
                                         lhsT=OT[:, h * RPC + j * 128:h * RPC + (j + 1) * 128],
                                         rhs=wo[l][h], start=(h != 0), stop=True,
                                         skip_group_check=True)
                        if h == 0:
                            nc.vector.tensor_scalar(out=accD[j][:], in0=wdp[:],
                                                    scalar1=rrc[:, j:j + 1],
                                                    scalar2=None, op0=ALU.mult)
                        else:
                            tmpw = wp.tile([128, D], F32, tag="tmpw")
                            nc.vector.tensor_scalar(out=tmpw[:], in0=wdp[:],
                                                    scalar1=rrc[:, h * 2 + j:h * 2 + j + 1],
                                                    scalar2=None, op0=ALU.mult)
                            nc.vector.tensor_tensor(out=accD[j][:], in0=accD[j][:],
                                                    in1=tmpw[:], op=ALU.add)
                    nc.vector.tensor_tensor(out=hpt[j][:], in0=hown[j][:],
                                            in1=accD[j][:], op=ALU.add)
                # LN2 + FF on own rows
                ln_group([hpt[j][:] for j in range(2)], l2g[l], l2b[l],
                         [xn2T[:, j * 128:(j + 1) * 128] for j in range(2)])
                for fs in range(4):
                    fp = pp.tile([128, RPC], F32, space="PSUM", tag="psA")
                    nc.tensor.matmul(out=fp[:], lhsT=w1[l][:, fs * 128:(fs + 1) * 128],
                                     rhs=xn2T[:], start=True, stop=True)
                    nc.scalar.activation(gT[fs][:], fp[:],
                                         AF.Gelu, bias=b1c[l][fs])
                fdp = pp.tile([D, RPC], F32, space="PSUM", tag="psA")
                for fs in range(4):
                    nc.tensor.matmul(out=fdp[:], lhsT=w2[l][fs],
                                     rhs=gT[fs][:],
                                     start=(fs == 0), stop=(fs == 3))
                ffdT = wp.tile([D, RPC], BF16, tag="ffdT")
                nc.scalar.activation(ffdT[:], fdp[:], AF.Identity, bias=b2c[l])
                for j in range(2):
                    ftp = pp.tile([128, 128], BF16, space="PSUM", tag="psA")
                    nc.tensor.transpose(out=ftp[:], in_=ffdT[:, j * 128:(j + 1) * 128],
                                        identity=ident[:])
                    nc.vector.tensor_tensor(out=hown[j][:], in0=hpt[j][:], in1=ftp[:],
                                            op=ALU.add)
                if DBG and l == 0:
                    for j in range(2):
                        dma(dbg["hL0"][j * 128:(j + 1) * 128, :], hown[j][:])
                # AllGather residual (not needed after last layer)
                if l < L - 1:
                    for j in range(2):
                        dma(ag_in[l][j * 128:(j + 1) * 128, :], hown[j][:])
                    nc.gpsimd.collective_compute(
                        "AllGather", ALU.bypass, replica_groups=[list(range(NC))],
                        ins=[ag_in[l].opt()], outs=[ag_out[l].opt()])
                    for t in range(NT):
                        dma(hfull[t][:], ag_out[l][t * 128:(t + 1) * 128, :])

            # ================= output =================
            for j in range(2):
                hb16 = wp.tile([128, D], BF16, tag="hb16")
                nc.vector.tensor_copy(hb16[:], hown[j][:])
                htp = pp.tile([128, 128], BF16, space="PSUM", tag="psA")
                nc.tensor.transpose(out=htp[:], in_=hb16[:], identity=ident[:])
                hT = wp.tile([D, 128], BF16, tag="hT")
                nc.vector.tensor_copy(hT[:], htp[:])
                op_ps = pp.tile([OUT, 128], F32, space="PSUM", tag="psA")
                nc.tensor.matmul(out=op_ps[:], lhsT=ow, rhs=hT[:],
                                 start=True, stop=True)
                ob_sb = wp.tile([OUT, 128], F32, tag="ob_sb")
                nc.scalar.activation(ob_sb[:], op_ps[:], AF.Identity,
                                     bias=obc[:, 0:1])
                dma(out_d[:, j * 128:(j + 1) * 128], ob_sb[:])

    nc.finalize()
    return nc


def _pack_bf16(I):
    Wq, Wk, Wo_, Wv_ = f32(I["Wq"]), f32(I["Wk"]), f32(I["Wo"]), f32(I["Wv"])
    cols = []
    for l in range(L):
        for h in range(H):
            cols.append(Wq[l, h])
    for l in range(L):
        for h in range(H):
            cols.append(Wk[l, h])
    for l in range(L):
        for h in range(H):
            cols.append(Wo_[l, h * D:(h + 1) * D, :])
    for l in range(L):
        cols.append(Wv_[l].transpose(1, 0, 2).reshape(D, H * D))
    for l in range(L):
        cols.append(f32(I["ff1_w"])[l])
    for l in range(L):
        for fs in range(4):
            cols.append(f32(I["ff2_w"])[l, fs * 128:(fs + 1) * 128, :])
    cols.append(f32(I["out_w"]))
    bvv = f32(I["bv"])
    for l in range(L):
        for h in range(H):
            cols.append(bvv[l, h][:, None])
    out = np.concatenate(cols, 1)
    assert out.shape == (128, NBF), out.shape
    return bf16(out)


def _pack_f32(I):
    cols = []
    for l in range(L):
        for h in range(H):
            cols.append(f32(I["bq"])[l, h][:, None])
    for l in range(L):
        for h in range(H):
            cols.append(f32(I["bk"])[l, h][:, None])
    for nm in ("ln1_g", "ln1_b", "ln2_g", "ln2_b"):
        for l in range(L):
            cols.append(f32(I[nm])[l][:, None])
    for l in range(L):
        cols.append(f32(I["ff2_b"])[l][:, None])
    for l in range(L):
        for fs in range(4):
            cols.append(f32(I["ff1_b"])[l, fs * 128:(fs + 1) * 128][:, None])
    out = np.concatenate(cols, 1)
    assert out.shape == (128, NF32), out.shape
    return f32(out)


def _prep(inputs):
    I = {k: np.asarray(v) for k, v in inputs.items()}
    x = f32(I["x"])
    pos = f32(I["pos"])
    ei = I["edge_index"].astype(np.int32)
    Wv = f32(I["Wv"])

    common = {
        "xa": bf16(np.concatenate([x.T, np.ones((1, N), np.float32)], 0)),
        "wina": bf16(np.concatenate([f32(I["node_in_w"]), f32(I["node_in_b"])[None]], 0)),
        "zin": bf16(I["z_in"]),
        "zout": bf16(I["z_out"]),
        "posT": bf16(pos.T),
        "wb": _pack_bf16(I),
        "pf": _pack_f32(I),
        "boall": f32(I["bo"]).reshape(1, L * D),
        "obc": f32(I["out_b"])[:, None],
        "spr": np.concatenate([f32(I["sp_mu"]), f32(I["sp_sigma"]), f32(I["sp_w"]),
                               np.full(HS, float(I["sp_b"]), np.float32)])[None].astype(np.float32),
        "iota64": np.arange(MAXD, dtype=np.float32)[:, None],
    }
    common = {k: np.ascontiguousarray(v) for k, v in common.items()}
    in_maps = []
    for c in range(NC):
        m = dict(common)
        qsl = slice(c * RPC, (c + 1) * RPC)
        m["pcT"] = bf16(pos[qsl].T)
        m["src"] = np.ascontiguousarray(
            ei[0, c * EPC:(c + 1) * EPC].reshape(EPC // 128, 128).T)
        m["dst"] = np.ascontiguousarray(
            ei[1, c * EPC:(c + 1) * EPC].reshape(EPC // 128, 128).T)
        m["oidx"] = np.ascontiguousarray(
            (c * RPC + np.arange(RPC, dtype=np.int32)).reshape(2, 128).T)
        in_maps.append(m)
    return in_maps


def kernel(**inputs) -> np.ndarray:
    if "nc" not in _cached:
        _cached["nc"] = build()
    in_maps = _prep(inputs)
    res = run_bass_kernel_spmd(_cached["nc"], in_maps, core_ids=list(range(NC)))
    _cached["last_results"] = res
    out = np.concatenate([f32(r["out"]).T for r in res.results], 0)
    return out.astype(np.float32)
